# revision 38
# baseline (speedup 1.0000x reference)
import os, sys
import numpy as np

sys.path.insert(0, "/opt/trn_rl_repo")

import concourse.bass as bass
import concourse.bacc as bacc
import concourse.tile as tile
import concourse.mybir as mybir
from concourse.bass_utils import run_bass_kernel_spmd

F32 = mybir.dt.float32
F32R = mybir.dt.float32r
BF16 = mybir.dt.bfloat16
F8 = mybir.dt.float8e4
AF = mybir.ActivationFunctionType
ALU = mybir.AluOpType

NC = 8
B, C, H, W = 64, 128, 28, 28
BL = B // NC
HW = H * W
T = BL * HW                  # 6272
HEADS, D = 4, 32
E = 512
KV, L = 15, 225
EPS = 1e-5
NG = float(B * HW)
SCALE = D ** -0.5
NT, TCH = 14, 448
KC0, KC1 = 128, L - 128

last_result = None


def _f32r(ap):
    return ap.bitcast(F32R)


def _class_ranges(k):
    if k == 0:
        return (1, 2)
    if k == 1:
        return (0, 1, 2)
    return (0, 1)


def _host_prep(inputs):
    import ml_dtypes
    bf = ml_dtypes.bfloat16
    f = lambda a: np.ascontiguousarray(np.asarray(a), dtype=np.float32)
    inp = {k: np.asarray(v) for k, v in inputs.items()}
    h = {}

    def diag(wk, ntap, dt):
        ch = wk.shape[0]
        nch = ch // 128
        out = np.zeros((128, nch, ntap, 128), dtype=np.float32)
        for cc in range(nch):
            for t in range(ntap):
                out[np.arange(128), cc, t, np.arange(128)] = wk[cc * 128:(cc + 1) * 128, t]
        return np.ascontiguousarray(out.astype(dt))

    import ml_dtypes as mld0
    f8t = mld0.float8_e4m3

    def diag_pairs(wk, pairs, dt, scale=16.0):
        # wk [ch, ntap]; returns [128, nch, npair, 2, 128]
        ch = wk.shape[0]
        nch = ch // 128
        out = np.zeros((128, nch, len(pairs), 2, 128), dtype=np.float32)
        for cc in range(nch):
            for pi, (ta, tb) in enumerate(pairs):
                out[np.arange(128), cc, pi, 0, np.arange(128)] = wk[cc * 128:(cc + 1) * 128, ta] * scale
                out[np.arange(128), cc, pi, 1, np.arange(128)] = wk[cc * 128:(cc + 1) * 128, tb] * scale
        return np.ascontiguousarray(out.astype(dt))

    # 3x3 taps indexed kh*3+kw; pairs chosen with constant in-pad stride:
    # (0,0)+(0,2)->S2, (2,0)+(2,2)->S2, (0,1)+(2,1)->S60, (1,0)+(1,2)->S2
    P9 = [(0, 2), (6, 8), (1, 7), (3, 5)]
    lpu9 = f(inp["lpu_w"]).reshape(C, 9)
    h["lpur"] = diag_pairs(lpu9, P9, f8t).reshape(128, 4, 2, 128)
    h["lpuc"] = diag((lpu9[:, 4:5] * 16.0), 1, f8t).reshape(128, 128)
    # 2x2 stride-2 taps kh*2+kw; pairs (0,0)+(0,1)->S1, (1,0)+(1,1)->S1
    P4 = [(0, 1), (2, 3)]
    h["kdwr"] = diag_pairs(f(inp["kdw_w"]).reshape(C, 4), P4, f8t).reshape(128, 2, 2, 128)
    h["vdwr"] = diag_pairs(f(inp["vdw_w"]).reshape(C, 4), P4, f8t).reshape(128, 2, 2, 128)
    h["wqT"] = f(inp["wq"]).T.copy().astype(bf)
    h["wkT"] = f(inp["wk"]).T.copy().astype(bf)
    h["wvT"] = f(inp["wv"]).T.copy().astype(bf)
    h["woT"] = f(inp["wo"]).T.copy().astype(bf)
    h["bq"] = f(inp["bq"]).reshape(C, 1)
    h["bkp"] = (f(inp["bk"]) + f(inp["wk"]) @ f(inp["kdw_b"])).reshape(C, 1)
    bvp = f(inp["bv"]) + f(inp["wv"]) @ f(inp["vdw_b"])
    h["bop"] = (f(inp["bo"]) + f(inp["wo"]) @ bvp + f(inp["lpu_b"])).reshape(C, 1)
    import ml_dtypes as mld
    f8 = mld.float8_e4m3
    # raw attention bias (divided by softmax scale), keys-major, for PSUM
    # preload ahead of the QK matmul: et = exp(SCALE*(qk + bias/SCALE))
    bq_ = f(inp["attn_bias"])[0].transpose(0, 2, 1) / (D ** -0.5)  # [4, 225, 784]
    bqp = np.zeros((128, 2, HEADS, HW), dtype=np.float32)
    bqp[:, 0] = bq_[:, 0:128, :].transpose(1, 0, 2)
    bqp[:KC1, 1] = bq_[:, 128:L, :].transpose(1, 0, 2)
    h["biasq"] = np.ascontiguousarray(bqp.astype(f8))
    eb = np.exp(f(inp["attn_bias"]))[0].transpose(0, 2, 1)  # [4, 225, 784]
    ebp = np.zeros((128, 2, HEADS, HW), dtype=np.float32)
    ebp[:, 0] = eb[:, 0:128, :].transpose(1, 0, 2)
    ebp[:KC1, 1] = eb[:, 128:L, :].transpose(1, 0, 2)
    h["expb"] = np.ascontiguousarray(ebp.astype(f8))
    idp = np.zeros((128, 128), dtype=np.float32)
    idp[np.arange(128), np.arange(128)] = 1.0
    h["identp"] = np.ascontiguousarray(idp.astype(f8))
    dww = f(inp["dw_w"]).reshape(E, 3, 3).copy()
    dww[:, 1, 1] += 1.0
    dwr9 = f(inp["dw_w"]).reshape(E, 9)
    h["fdr"] = diag_pairs(dwr9, P9, f8t)                 # [128, 4, 4, 2, 128]
    h["fctr"] = diag((dwr9[:, 4:5] + 1.0) * 16.0, 1, bf).reshape(128, 4, 128)
    h["dw_b"] = f(inp["dw_b"]).reshape(4, 128).T.copy()
    psum9 = np.zeros((9, 4, 128), dtype=np.float32)
    for k in range(9):
        hr, wr = _class_ranges(k // 3), _class_ranges(k % 3)
        s = dww[:, hr, :][:, :, wr].sum(axis=(1, 2))
        psum9[k] = s.reshape(4, 128)
    h["psum9"] = psum9 * 16.0
    ind9 = np.zeros((9, H, W), dtype=np.float32)
    hc = np.full(H, 1); hc[0] = 0; hc[-1] = 2
    wc = np.full(W, 1); wc[0] = 0; wc[-1] = 2
    for i in range(H):
        for j in range(W):
            ind9[hc[i] * 3 + wc[j], i, j] = 1.0
    h["ind9"] = ind9.reshape(9, HW).astype(bf)
    h["c1wT"] = f(inp["c1_w"]).T.copy().astype(bf)
    h["c1_b"] = f(inp["c1_b"]).reshape(4, 128).T.copy()
    h["w2T"] = f(inp["c2_w"]).T.reshape(4, 128, 128).transpose(1, 0, 2).copy().astype(bf)
    h["bn1_g"] = f(inp["bn1_g"]).reshape(4, 128).T.copy()
    h["bn1_b"] = f(inp["bn1_b"]).reshape(4, 128).T.copy()
    h["bnr_g"] = f(inp["bnr_g"]).reshape(4, 128).T.copy()
    h["bnr_b"] = f(inp["bnr_b"]).reshape(4, 128).T.copy()
    h["bn2_g"] = f(inp["bn2_g"]).reshape(C, 1)
    h["bn2_b"] = f(inp["bn2_b"]).reshape(C, 1)
    ln_triv = (np.allclose(inp["ln1_g"], 1) and np.allclose(inp["ln1_b"], 0)
               and np.allclose(inp["ln2_g"], 1) and np.allclose(inp["ln2_b"], 0))
    h["_ln_triv"] = ln_triv
    if not ln_triv:
        h["ln1_g"] = f(inp["ln1_g"]).reshape(1, HW)
        h["ln1_b"] = f(inp["ln1_b"]).reshape(1, HW)
        h["ln2_g"] = f(inp["ln2_g"]).reshape(1, HW)
        h["ln2_b"] = f(inp["ln2_b"]).reshape(1, HW)
    return h


def _build(ln_triv):
    nc = bacc.Bacc(None, target_bir_lowering=False, num_devices=NC)
    dt = nc.dram_tensor
    xs = dt("xs", [BL, C, H, W], F32, kind="ExternalInput")
    out_t = dt("out", [BL, C, H, W], F32, kind="ExternalOutput")
    hin = {}
    specs = [
        ("lpur", [128, 4, 2, 128], F8), ("lpuc", [128, 128], F8),
        ("kdwr", [128, 2, 2, 128], F8), ("vdwr", [128, 2, 2, 128], F8),
        ("fdr", [128, 4, 4, 2, 128], F8), ("fctr", [128, 4, 128], BF16),
        ("wqT", [C, C], BF16), ("wkT", [C, C], BF16), ("wvT", [C, C], BF16),
        ("woT", [C, C], BF16), ("bq", [C, 1], F32), ("bkp", [C, 1], F32),
        ("bop", [C, 1], F32), ("biasq", [128, 2, HEADS, HW], F8),
        ("identp", [C, C], F8),
        ("dw_b", [128, 4], F32),
        ("psum9", [9, 4, 128], F32), ("ind9", [9, HW], BF16),
        ("c1wT", [C, E], BF16), ("c1_b", [128, 4], F32),
        ("w2T", [128, 4, 128], BF16),
        ("bn1_g", [128, 4], F32), ("bn1_b", [128, 4], F32),
        ("bnr_g", [128, 4], F32), ("bnr_b", [128, 4], F32),
        ("bn2_g", [C, 1], F32), ("bn2_b", [C, 1], F32),
    ]
    if not ln_triv:
        specs += [(n, [1, HW], F32) for n in ["ln1_g", "ln1_b", "ln2_g", "ln2_b"]]
    for name, shape, d in specs:
        hin[name] = dt(name, shape, d, kind="ExternalInput")
    ar_in = {0: dt("ar0i", [128, 1], F32, kind="Internal"),
             1: dt("ar1i", [128, 8], F32, kind="Internal"),
             2: dt("ar2i", [128, 8], F32, kind="Internal"),
             3: dt("ar3i", [128, 2], F32, kind="Internal")}
    ar_out = {0: dt("ar0o", [128, 1], F32, kind="Internal", addr_space="Shared"),
              1: dt("ar1o", [128, 8], F32, kind="Internal", addr_space="Shared"),
              2: dt("ar2o", [128, 8], F32, kind="Internal", addr_space="Shared"),
              3: dt("ar3o", [128, 2], F32, kind="Internal", addr_space="Shared")}
    c1_dram = dt("c1d", [128, 4], F32, kind="Internal")
    RG = [list(range(NC))]
    with tile.TileContext(nc) as tc:
        _emit(nc, tc, xs, out_t, hin, ar_in, ar_out, c1_dram, RG, ln_triv)
    if not nc.is_finalized():
        nc.finalize()
    return nc


def _emit(nc, tc, xs, out_t, hin, ar_in, ar_out, c1_dram, RG, ln_triv):
    from contextlib import ExitStack
    ctx = ExitStack()
    with ctx:
        big = ctx.enter_context(tc.tile_pool(name="big", bufs=2))
        bfp = ctx.enter_context(tc.tile_pool(name="bfp", bufs=1))
        cons = ctx.enter_context(tc.tile_pool(name="cons", bufs=1))
        small = ctx.enter_context(tc.tile_pool(name="small", bufs=1))
        etp = ctx.enter_context(tc.tile_pool(name="etp", bufs=8))
        psA = ctx.enter_context(tc.tile_pool(name="psA", bufs=4, space="PSUM"))
        psB = ctx.enter_context(tc.tile_pool(name="psB", bufs=2, space="PSUM"))
        psC = ctx.enter_context(tc.tile_pool(name="psC", bufs=2, space="PSUM"))

        def loadc(name):
            hh = hin[name]
            t = cons.tile(list(hh.shape), hh.dtype, tag=name)
            nc.gpsimd.dma_start(out=t, in_=hh[:])
            return t

        nc.gpsimd.collective_compute("AllReduce", ALU.add, RG,
                                     ins=[ar_in[0][:]], outs=[ar_out[0][:]])
        lpur = loadc("lpur"); lpuc = loadc("lpuc")
        kdwr = loadc("kdwr"); vdwr = loadc("vdwr")
        fdr = loadc("fdr"); fctr = loadc("fctr")
        wqT = loadc("wqT"); wkT = loadc("wkT"); wvT = loadc("wvT"); woT = loadc("woT")
        bq = loadc("bq"); bkp = loadc("bkp"); bop = loadc("bop")
        biasq = loadc("biasq"); identp = loadc("identp")
        dw_b = loadc("dw_b")
        psum9 = loadc("psum9"); ind9t = loadc("ind9")
        c1wT = loadc("c1wT"); c1_b = loadc("c1_b"); w2T = loadc("w2T")
        bn1_g = loadc("bn1_g"); bn1_b = loadc("bn1_b")
        bnr_g = loadc("bnr_g"); bnr_b = loadc("bnr_b")
        bn2_g = loadc("bn2_g"); bn2_b = loadc("bn2_b")
        ind9 = ind9t.rearrange("k (h w) -> k h w", h=H)
        lns = {}
        if not ln_triv:
            for nm in ["ln1_g", "ln1_b", "ln2_g", "ln2_b"]:
                t = cons.tile([128, HW], F32, tag=nm)
                nc.gpsimd.dma_start(out=t, in_=bass.AP(tensor=hin[nm], offset=0, ap=[[0, 128], [1, HW]]))
                lns[nm] = t
        epsT = small.tile([128, 1], F32, tag="epsT")
        nc.vector.memset(epsT, EPS)
        # pre-touch DMA-loaded consts on the engines that read them, so heavy
        # ops don't accumulate multiple DMA-queue sem waits (codegen limit)
        scrD = small.tile([128, 1], F32, tag="scrD")
        scrA = small.tile([128, 1], F32, tag="scrA")
        for t2 in (bq, bkp, bop, bn2_g, bn2_b):
            nc.vector.tensor_copy(out=scrD, in_=t2[:, 0:1])
        nc.vector.tensor_copy(out=scrD, in_=lpur[:, 0, 0, 0:1])
        nc.vector.tensor_copy(out=scrD, in_=kdwr[:, 0, 0, 0:1])
        nc.vector.tensor_copy(out=scrD, in_=vdwr[:, 0, 0, 0:1])
        nc.vector.tensor_copy(out=scrD, in_=w2T[:, 0, 0:1])
        nc.vector.tensor_copy(out=scrD, in_=lpuc[:, 0:1])
        nc.vector.tensor_copy(out=scrD, in_=fdr[:, 0, 0, 0, 0:1])
        nc.vector.tensor_copy(out=scrD, in_=fctr[:, 0, 0:1])
        for t4 in (wqT, wkT, wvT, woT, c1wT):
            nc.vector.tensor_copy(out=scrD, in_=t4[:, 0:1])
        nc.vector.tensor_copy(out=scrD, in_=biasq[:, 0, 0, 0:1])
        nc.vector.tensor_copy(out=scrD, in_=identp[:, 0:1])
        for t5 in (dw_b, c1_b, bn1_g, bn1_b, bnr_g, bnr_b):
            nc.vector.tensor_copy(out=scrD, in_=t5[:, 0:1])
        nc.vector.tensor_copy(out=scrD[0:9], in_=psum9[:, 0, 0:1])
        nc.vector.tensor_copy(out=scrD[0:9], in_=ind9t[:, 0:1])
        for t6 in lns.values():
            nc.vector.tensor_copy(out=scrD, in_=t6[:, 0:1])
        nc.scalar.mul(out=scrA, in_=c1_b[:, 0:1], mul=1.0)
        nc.scalar.mul(out=scrA, in_=dw_b[:, 0:1], mul=1.0)

        xsb = big.tile([128, BL, HW], F32, tag="big")
        nc.gpsimd.dma_start(out=xsb, in_=xs[:].rearrange("b c h w -> c b (h w)"))
        nc.vector.tensor_copy(out=scrD, in_=xsb[:, 0, 0:1])
        # zero-padded fp8 copy of x: [128, BL, 30, 30], image at [1:29, 1:29]
        xpad = bfp.tile([128, BL, 30, 30], F8, tag="pad8")
        nc.vector.memset(xpad[:, :, 0, :], 0.0)
        nc.vector.memset(xpad[:, :, 29, :], 0.0)
        nc.vector.memset(xpad[:, :, 1:29, 0], 0.0)
        nc.vector.memset(xpad[:, :, 1:29, 29], 0.0)
        with nc.allow_low_precision("conv input in fp8"):
            xsb4v = xsb.rearrange("p b (h w) -> p b h w", h=H)
            for b in range(BL):
                nc.gpsimd.tensor_copy(out=xpad[:, b, 1:29, 1:29], in_=xsb4v[:, b])

        # x16-scaled fp8 DoubleRow taps; psum holds 16*dw(x); the +x residual
        # and /16 happen at evac.  Pair t reads (khA,kwA)/(khB,kwB) windows of
        # the padded image via an overlapping stride-S access pattern.
        P9T = [((0, 0), (0, 2)), ((2, 0), (2, 2)), ((0, 1), (2, 1)), ((1, 0), (1, 2))]

        def pad_pair_rhs(padt, pre, b, base, pa, pb_):
            (ka, wa), (kb, wb) = pa, pb_
            S = (kb - ka) * 30 + (wb - wa)
            off = padt.offset + (pre + b) * 900 + (base + ka) * 30 + wa
            return bass.AP(tensor=padt.tensor, offset=off,
                           ap=[list(padt.ap[0]), [S, 2], [30, 14], [1, 28]])

        def pad_tap_rhs(padt, pre, b, base, kh, kw):
            off = padt.offset + (pre + b) * 900 + (base + kh) * 30 + kw
            return bass.AP(tensor=padt.tensor, offset=off,
                           ap=[list(padt.ap[0]), [30, 14], [1, 28]])

        x_lpu = big.tile([128, BL, HW], F32, tag="big")
        xlp4 = x_lpu.rearrange("p b (h w) -> p b h w", h=H)
        xsb4 = xsb.rearrange("p b (h w) -> p b h w", h=H)

        for half in range(2):
            base = 14 * half
            pts = []
            for b in range(BL):
                pool = (psA, psA, psA, psA, psB, psB, psC, psC)[b]
                tg = ("mm", "mm", "mm", "mm", "dwp", "dwp", "av", "av")[b]
                pts.append(pool.tile([128, 14, W], F32, tag=tg, name=f"lvp{b}"))
            for pi in range(4):
                pa, pb_ = P9T[pi]
                for b in range(BL):
                    nc.tensor.matmul(
                        pts[b], lpur[:, pi], pad_pair_rhs(xpad, 0, b, base, pa, pb_),
                        start=(pi == 0), stop=False,
                        perf_mode=mybir.MatmulPerfMode.DoubleRow,
                        skip_group_check=True)
            for b in range(BL):
                nc.tensor.matmul(
                    pts[b], lpuc, pad_tap_rhs(xpad, 0, b, base, 1, 1),
                    start=False, stop=True, skip_group_check=True)
            for b in range(BL):
                nc.vector.scalar_tensor_tensor(
                    out=xlp4[:, b, base:base + 14, :], in0=pts[b], scalar=1.0 / 16.0,
                    in1=xsb4[:, b, base:base + 14, :], op0=ALU.mult, op1=ALU.add)

        # LN over HW
        def layer_norm(src, gname, dst):
            sv = src.rearrange("p b (two q) -> p b two q", two=2)
            st = small.tile([128, BL, 2, 6], F32, tag="lnst")
            mv = small.tile([128, BL, 2], F32, tag="lnmv")
            sd = small.tile([128, BL, 1], F32, tag="lnsd")
            for b in range(BL):
                for g2 in range(2):
                    nc.vector.bn_stats(out=st[:, b, g2], in_=sv[:, b, g2])
                nc.vector.bn_aggr(out=mv[:, b], in_=st[:, b])
            nc.scalar.activation(out=sd, in_=mv[:, :, 1:2], func=AF.Sqrt, bias=epsT, scale=1.0)
            nc.vector.reciprocal(out=sd, in_=sd)
            for b in range(BL):
                nc.vector.tensor_scalar(
                    out=dst[:, b], in0=src[:, b], scalar1=mv[:, b, 0:1], scalar2=sd[:, b],
                    op0=ALU.subtract, op1=ALU.mult)
            if not ln_triv:
                g = lns[gname + "_g"]; bb = lns[gname + "_b"]
                for b in range(BL):
                    nc.vector.tensor_mul(out=dst[:, b], in0=dst[:, b], in1=g)
                    nc.vector.tensor_add(out=dst[:, b], in0=dst[:, b], in1=bb)

        xnbf = bfp.tile([128, BL, HW], BF16, tag="t12b")
        layer_norm(x_lpu, "ln1", xnbf)
        xnpad = bfp.tile([128, BL, 30, 30], F8, tag="pad8")
        nc.vector.memset(xnpad[:, :, 0, :], 0.0)
        nc.vector.memset(xnpad[:, :, 29, :], 0.0)
        nc.vector.memset(xnpad[:, :, 1:29, 0], 0.0)
        nc.vector.memset(xnpad[:, :, 1:29, 29], 0.0)
        with nc.allow_low_precision("conv input in fp8"):
            xnbf4v = xnbf.rearrange("p b (h w) -> p b h w", h=H)
            for b in range(BL):
                nc.gpsimd.tensor_copy(out=xnpad[:, b, 1:29, 1:29], in_=xnbf4v[:, b])

        # Q projection (f32r) -> bf16
        qbf = bfp.tile([128, BL, HW], BF16, tag="qbf")
        xnbff = xnbf.rearrange("p b q -> p (b q)")
        qbff = qbf.rearrange("p b q -> p (b q)")
        for i in range(NT):
            pt = psA.tile([128, TCH], F32, tag="mm")
            nc.tensor.matmul(pt, wqT, xnbff[:, i * TCH:(i + 1) * TCH], start=True, stop=True)
            nc.vector.tensor_scalar(out=qbff[:, i * TCH:(i + 1) * TCH], in0=pt, scalar1=bq,
                                    scalar2=None, op0=ALU.add)
        # K/V strided 2x2 dw conv
        kxbf = bfp.tile([128, BL, L], BF16, tag="kxbf")
        vxbf = bfp.tile([128, BL, L], BF16, tag="vxbf")
        kx4 = kxbf.rearrange("p b (i j) -> p b i j", i=KV)
        vx4 = vxbf.rearrange("p b (i j) -> p b i j", i=KV)
        # kv conv: out(i,j) = sum w[kh,kw]*xn[2i+kh-1, 2j+kw-1]
        #        = sum w[kh,kw]*xnpad[2i+kh, 2j+kw], i,j in [0,15)
        def kv_pair_rhs(b, kh):
            off = xnpad.offset + b * 900 + kh * 30
            return bass.AP(tensor=xnpad.tensor, offset=off,
                           ap=[list(xnpad.ap[0]), [1, 2], [60, KV], [2, KV]])

        for b in range(BL):
            for dst4, dg in ((kx4, kdwr), (vx4, vdwr)):
                pt = psA.tile([128, KV, KV], F32, tag="mm")
                for kh in range(2):
                    nc.tensor.matmul(
                        pt, dg[:, kh], kv_pair_rhs(b, kh),
                        start=(kh == 0), stop=(kh == 1),
                        perf_mode=mybir.MatmulPerfMode.DoubleRow,
                        skip_group_check=True)
                nc.scalar.activation(out=dst4[:, b], in_=pt, func=AF.Copy,
                                     scale=1.0 / 16.0)
        kbf = bfp.tile([128, BL, L], BF16, tag="kbf")
        kxf = kxbf.rearrange("p b l -> p (b l)")
        kbff = kbf.rearrange("p b l -> p (b l)")
        for i in range(4):
            pt = psA.tile([128, 450], F32, tag="mm")
            nc.tensor.matmul(pt, wkT, kxf[:, i * 450:(i + 1) * 450], start=True, stop=True)
            nc.vector.tensor_scalar(out=kbff[:, i * 450:(i + 1) * 450], in0=pt, scalar1=bkp,
                                    scalar2=None, op0=ALU.add)
        vaug = bfp.tile([128, BL, 2, HEADS, 64], F8, tag="vaug")
        nc.vector.memset(vaug, 0.0)
        nc.vector.memset(vaug[:, :, :, :, 32:64], 1.0)
        for b in range(BL):
            for kc in range(2):
                ktM = KC0 if kc == 0 else KC1
                pt = psA.tile([128, 128], F32, tag="mm")
                nc.tensor.matmul(pt[0:ktM], vxbf[:, b, kc * 128: kc * 128 + ktM], wvT,
                                 start=True, stop=True)
                with nc.allow_low_precision("attention V in fp8"):
                    nc.scalar.copy(out=vaug[0:ktM, b, kc, :, 0:32],
                                   in_=pt[0:ktM].rearrange("p (h d) -> p h d", h=HEADS))

        # attention: QK into psum (4 heads row-tiled), then the attention bias
        # preloaded on top via one full identity matmul; exp at evac includes
        # the bias; AV with a ones-block for denominators; fast-approx
        # reciprocal of the denominators.
        o_sb = bfp.tile([128, BL, HW], BF16, tag="t12a")
        rbc = bfp.tile([128, BL, HW], F32, tag="h1h2")
        for b in range(BL):
            dscr = bfp.tile([128, HW], F32, tag="kxbf", name=f"dscr{b}")
            for qc in range(2):
                qs = slice(qc * 392, (qc + 1) * 392)
                ets = {}
                for kc in range(2):
                    ktM = KC0 if kc == 0 else KC1
                    pts = []
                    for hd in range(HEADS):
                        pt = psA.tile([128, 392], F32, tag="mm")
                        nc.tensor.matmul(
                            pt[0:ktM],
                            kbf[hd * 32:(hd + 1) * 32, b, kc * 128: kc * 128 + ktM],
                            qbf[hd * 32:(hd + 1) * 32, b, qs],
                            start=True, stop=False, tile_position=(hd * 32, 0),
                            skip_group_check=True)
                        pts.append(pt)
                    for hd in range(HEADS):
                        pt = pts[hd]
                        nc.tensor.matmul(
                            pt, identp, biasq[:, kc, hd, qs],
                            start=False, stop=True, skip_group_check=True)
                        et = etp.tile([128, 392], F8, tag="et")
                        with nc.allow_low_precision("attention scores fp8"):
                            nc.scalar.activation(out=et[0:ktM], in_=pt[0:ktM],
                                                 func=AF.Exp, scale=SCALE)
                        ets[(kc, hd)] = et
                for hp in range(2):
                    pv = psC.tile([128, 392], F32, tag="av")
                    for kc in range(2):
                        ktM = KC0 if kc == 0 else KC1
                        for hh in range(2):
                            hd = hp * 2 + hh
                            nc.tensor.matmul(
                                pv[64 * hh:64 * hh + 64],
                                vaug[0:ktM, b, kc, hd, :], ets[(kc, hd)][0:ktM],
                                start=(kc == 0), stop=(kc == 1),
                                tile_position=(0, 64 * hh), skip_group_check=True)
                    for hh in range(2):
                        hd = hp * 2 + hh
                        with nc.allow_low_precision("attention numerators bf16"):
                            nc.vector.tensor_copy(
                                out=o_sb[hd * 32:(hd + 1) * 32, b, qs],
                                in_=pv[64 * hh:64 * hh + 32])
                        nc.scalar.copy(
                            out=dscr[hd * 32:(hd + 1) * 32, qs],
                            in_=pv[64 * hh + 32:64 * hh + 64])
                # fast reciprocal of this (b, qc)'s denominators (packed dup
                # layout): magic seed + one Newton, standard ops.  rbc ends
                # NEGATED ((x*y0-2)*y0 = -1/x); the o-mul flips the sign.
                nc.vector.tensor_scalar(
                    out=rbc[:, b, qs].bitcast(mybir.dt.int32),
                    in0=dscr[:, qs].bitcast(mybir.dt.int32),
                    scalar1=-1, scalar2=0x7EF127EA, op0=ALU.mult, op1=ALU.add)
                nc.vector.tensor_mul(out=dscr[:, qs], in0=dscr[:, qs], in1=rbc[:, b, qs])
                nc.vector.scalar_tensor_tensor(
                    out=rbc[:, b, qs], in0=dscr[:, qs], scalar=2.0, in1=rbc[:, b, qs],
                    op0=ALU.subtract, op1=ALU.mult)
        nc.vector.scalar_tensor_tensor(out=o_sb, in0=o_sb, scalar=-1.0, in1=rbc,
                                       op0=ALU.mult, op1=ALU.mult)

        x_mhsa = big.tile([128, BL, HW], F32, tag="big")
        of = o_sb.rearrange("p b q -> p (b q)")
        xmf = x_mhsa.rearrange("p b q -> p (b q)")
        xlf = x_lpu.rearrange("p b q -> p (b q)")
        for i in range(NT):
            pt = psA.tile([128, TCH], F32, tag="mm")
            nc.tensor.matmul(pt, woT, of[:, i * TCH:(i + 1) * TCH], start=True, stop=True)
            nc.vector.scalar_tensor_tensor(out=xmf[:, i * TCH:(i + 1) * TCH], in0=pt, scalar=bop,
                                           in1=xlf[:, i * TCH:(i + 1) * TCH], op0=ALU.add, op1=ALU.add)

        ybf = bfp.tile([128, BL, HW], BF16, tag="t12a")
        layer_norm(x_mhsa, "ln2", ybf)

        def bn_reduce(src_r, nchunk, ar_i, ar_o):
            # global batch statistics: local bn_stats/aggr, then a cross-core
            # AllReduce of (mean*T, (mean^2+var)*T)
            st = small.tile([128, nchunk, NT, 6], F32, tag="bnst")
            mv = small.tile([128, nchunk, 2], F32, tag="bnmv")
            for ecx in range(nchunk):
                for i in range(NT):
                    nc.vector.bn_stats(out=st[:, ecx, i], in_=src_r[:, ecx, i])
                nc.vector.bn_aggr(out=mv[:, ecx], in_=st[:, ecx])
            stats = small.tile([128, nchunk, 2], F32, tag="bnpack")
            m2 = small.tile([128, nchunk], F32, tag="bnm2")
            nc.vector.tensor_scalar(out=stats[:, :, 0:1], in0=mv[:, :, 0:1], scalar1=float(T),
                                    scalar2=None, op0=ALU.mult)
            nc.vector.tensor_mul(out=m2, in0=mv[:, :, 0], in1=mv[:, :, 0])
            nc.vector.tensor_add(out=m2, in0=m2, in1=mv[:, :, 1])
            nc.vector.tensor_scalar(out=stats[:, :, 1:2], in0=m2.rearrange("p (e o) -> p e o", o=1),
                                    scalar1=float(T), scalar2=None, op0=ALU.mult)
            nc.gpsimd.dma_start(out=ar_i[:], in_=stats.rearrange("p e two -> p (e two)"))
            nc.gpsimd.collective_compute("AllReduce", ALU.add, RG, ins=[ar_i[:]], outs=[ar_o[:]])
            g = small.tile([128, nchunk, 2], F32, tag="bngl")
            nc.gpsimd.dma_start(out=g.rearrange("p e two -> p (e two)"), in_=ar_o[:])
            return g

        def bn_affine(gs, nchunk, gt, bt, want_coa=False):
            a = small.tile([128, nchunk], F32, tag="bna")
            cc = small.tile([128, nchunk], F32, tag="bnc")
            mean = small.tile([128, nchunk], F32, tag="bnmean")
            m2 = small.tile([128, nchunk], F32, tag="bnm2b")
            nc.vector.tensor_scalar(out=mean, in0=gs[:, :, 0], scalar1=1.0 / NG, scalar2=None, op0=ALU.mult)
            nc.vector.tensor_scalar(out=a, in0=gs[:, :, 1], scalar1=1.0 / NG, scalar2=None, op0=ALU.mult)
            nc.vector.tensor_mul(out=m2, in0=mean, in1=mean)
            nc.vector.tensor_sub(out=a, in0=a, in1=m2)
            nc.scalar.activation(out=a, in_=a, func=AF.Sqrt, bias=epsT, scale=1.0)
            nc.vector.reciprocal(out=a, in_=a)
            nc.vector.tensor_mul(out=a, in0=a, in1=gt)
            nc.vector.tensor_mul(out=cc, in0=mean, in1=a)
            nc.vector.scalar_tensor_tensor(out=cc, in0=cc, scalar=-1.0, in1=bt,
                                           op0=ALU.mult, op1=ALU.add)
            if not want_coa:
                return a, cc
            ra = small.tile([128, nchunk], F32, tag="bnra")
            coa = small.tile([128, nchunk], F32, tag="bncoa")
            nc.vector.reciprocal(out=ra, in_=a)
            nc.vector.tensor_mul(out=coa, in0=cc, in1=ra)
            return a, cc, coa

        # pw1 + gelu -> h1bf
        h1bf = bfp.tile([128, 4, BL, HW], BF16, tag="h1h2")
        h1f = h1bf.rearrange("p e b q -> p e (b q)")
        ybff = ybf.rearrange("p b q -> p (b q)")
        for ec in range(4):
            for i in range(NT):
                pt = psA.tile([128, TCH], F32, tag="mm")
                nc.tensor.matmul(pt, c1wT[:, ec * 128:(ec + 1) * 128],
                                 ybff[:, i * TCH:(i + 1) * TCH], start=True, stop=True)
                nc.scalar.activation(out=h1f[:, ec, i * TCH:(i + 1) * TCH], in_=pt, func=AF.Gelu,
                                     bias=c1_b[:, ec:ec + 1], scale=1.0)
        # BN1: local stats feed the border-correction weights (lh9) so the
        # conv pipeline never waits on a collective; the gelu-evac scale/bias
        # (a1, via a1o16) uses the exact global stats from the AllReduce.
        st1l = small.tile([128, 4, NT, 6], F32, tag="bnst")
        mv1l = small.tile([128, 4, 2], F32, tag="bnmv")
        h1r = h1f.rearrange("p e (n q) -> p e n q", q=TCH)
        for ecx in range(4):
            for i in range(NT):
                nc.vector.bn_stats(out=st1l[:, ecx, i], in_=h1r[:, ecx, i])
            nc.vector.bn_aggr(out=mv1l[:, ecx], in_=st1l[:, ecx])
        stats1 = small.tile([128, 4, 2], F32, tag="bnpack")
        m2l = small.tile([128, 4], F32, tag="bnm2l")
        nc.vector.tensor_scalar(out=stats1[:, :, 0:1], in0=mv1l[:, :, 0:1],
                                scalar1=float(T), scalar2=None, op0=ALU.mult)
        nc.vector.tensor_mul(out=m2l, in0=mv1l[:, :, 0], in1=mv1l[:, :, 0])
        nc.vector.tensor_add(out=m2l, in0=m2l, in1=mv1l[:, :, 1])
        nc.vector.tensor_scalar(out=stats1[:, :, 1:2], in0=m2l.rearrange("p (e o) -> p e o", o=1),
                                scalar1=float(T), scalar2=None, op0=ALU.mult)
        nc.gpsimd.dma_start(out=ar_in[1][:], in_=stats1.rearrange("p e two -> p (e two)"))
        nc.gpsimd.collective_compute("AllReduce", ALU.add, RG,
                                     ins=[ar_in[1][:]], outs=[ar_out[1][:]])
        gs1l = small.tile([128, 4, 2], F32, tag="bngl1")
        nc.vector.tensor_scalar(out=gs1l, in0=stats1, scalar1=float(NG / T),
                                scalar2=None, op0=ALU.mult)
        _a1l, _c1l, coa1 = bn_affine(gs1l, 4, bn1_g, bn1_b, want_coa=True)
        nc.gpsimd.dma_start(out=c1_dram[:], in_=coa1)
        c1row = small.tile([1, 4, 128], F32, tag="c1row")
        nc.gpsimd.dma_start(out=c1row, in_=bass.AP(tensor=c1_dram, offset=0, ap=[[0, 1], [1, 4], [4, 128]]))
        c1f = small.tile([9, 4, 128], F32, tag="c1f")
        nc.gpsimd.partition_broadcast(c1f, c1row)
        lh9 = small.tile([9, 4, 128], BF16, tag="lh9")
        nc.vector.tensor_mul(out=lh9, in0=psum9, in1=c1f)
        g1 = small.tile([128, 4, 2], F32, tag="bngl")
        nc.gpsimd.dma_start(out=g1.rearrange("p e two -> p (e two)"), in_=ar_out[1][:])
        a1, c1 = bn_affine(g1, 4, bn1_g, bn1_b)
        a1o16 = small.tile([128, 4], F32, tag="a1o16")
        nc.vector.tensor_scalar(out=a1o16, in0=a1, scalar1=1.0 / 16.0, scalar2=None,
                                op0=ALU.mult)

        # FFN dw via x16 fp8 pairs + fp8 center + bf16 identity (the +h1
        # residual) + border-count correction; BN1 scale a1 applied at evac.
        # The padded fp8 copy of h1 is built per-ec (ring of 2) to fit SBUF.
        h2g = h1bf
        h2g4 = h2g.rearrange("p e b (h w) -> p e b h w", h=H)
        h1b4 = h1bf.rearrange("p e b (h w) -> p e b h w", h=H)
        h2f = h2g.rearrange("p e b q -> p e (b q)")
        h2r = h2f.rearrange("p e (n q) -> p e n q", q=TCH)
        st2 = small.tile([128, 4, NT, 6], F32, tag="bnst")
        mv2 = small.tile([128, 4, 2], F32, tag="bnmv")
        for ec in range(4):
            gfp8 = bfp.tile([128, BL, 30, 30], F8, tag="gf8", bufs=2, name=f"gch{ec}")
            nc.vector.memset(gfp8[:, :, 0, :], 0.0)
            nc.vector.memset(gfp8[:, :, 29, :], 0.0)
            nc.vector.memset(gfp8[:, :, 1:29, 0], 0.0)
            nc.vector.memset(gfp8[:, :, 1:29, 29], 0.0)
            with nc.allow_low_precision("conv input in fp8"):
                nc.vector.tensor_copy(
                    out=gfp8[:, :, 1:29, 1:29],
                    in_=h1b4[:, ec])
            for half in range(2):
                base = 14 * half
                pts = []
                for b in range(BL):
                    pool = (psA, psA, psA, psA, psB, psB, psC, psC)[b]
                    tg = ("mm", "mm", "mm", "mm", "dwp", "dwp", "av", "av")[b]
                    pts.append(pool.tile([128, 14, W], F32, tag=tg, name=f"cvp{b}"))
                for pi in range(4):
                    pa, pb_ = P9T[pi]
                    for b in range(BL):
                        nc.tensor.matmul(
                            pts[b], fdr[:, ec, pi],
                            pad_pair_rhs(gfp8, 0, b, base, pa, pb_),
                            start=(pi == 0), stop=False,
                            perf_mode=mybir.MatmulPerfMode.DoubleRow,
                            skip_group_check=True)
                for b in range(BL):
                    nc.tensor.matmul(
                        pts[b], fctr[:, ec], h1b4[:, ec, b, base:base + 14, :],
                        start=False, stop=False, skip_group_check=True)
                for b in range(BL):
                    nc.tensor.matmul(pts[b], lh9[:, ec], ind9[:, base:base + 14, :],
                                     start=False, stop=True, skip_group_check=True)
                for b in range(BL):
                    nc.scalar.activation(out=h2g4[:, ec, b, base:base + 14, :], in_=pts[b],
                                         func=AF.Gelu, bias=dw_b[:, ec:ec + 1],
                                         scale=a1o16[:, ec:ec + 1])
                if half == 1:
                    # per-ec stats as soon as this ec's evacs land, so only
                    # the AllReduce tail is exposed after the conv
                    for i in range(NT):
                        nc.vector.bn_stats(out=st2[:, ec, i], in_=h2r[:, ec, i])
                    nc.vector.bn_aggr(out=mv2[:, ec], in_=st2[:, ec])
        stats2 = small.tile([128, 4, 2], F32, tag="bnpack")
        m2b = small.tile([128, 4], F32, tag="bnm2")
        nc.vector.tensor_scalar(out=stats2[:, :, 0:1], in0=mv2[:, :, 0:1],
                                scalar1=float(T), scalar2=None, op0=ALU.mult)
        nc.vector.tensor_mul(out=m2b, in0=mv2[:, :, 0], in1=mv2[:, :, 0])
        nc.vector.tensor_add(out=m2b, in0=m2b, in1=mv2[:, :, 1])
        nc.vector.tensor_scalar(out=stats2[:, :, 1:2], in0=m2b.rearrange("p (e o) -> p e o", o=1),
                                scalar1=float(T), scalar2=None, op0=ALU.mult)
        nc.gpsimd.dma_start(out=ar_in[2][:], in_=stats2.rearrange("p e two -> p (e two)"))
        nc.gpsimd.collective_compute("AllReduce", ALU.add, RG,
                                     ins=[ar_in[2][:]], outs=[ar_out[2][:]])
        g2 = small.tile([128, 4, 2], F32, tag="bngl")
        nc.gpsimd.dma_start(out=g2.rearrange("p e two -> p (e two)"), in_=ar_out[2][:])
        a2, c2 = bn_affine(g2, 4, bnr_g, bnr_b)
        w2s = bfp.tile([128, 4, 128], BF16, tag="t12a")
        for kc in range(4):
            nc.vector.tensor_scalar(out=w2s[:, kc], in0=w2T[:, kc], scalar1=a2[:, kc:kc + 1],
                                    scalar2=None, op0=ALU.mult)
        c2bf = small.tile([128, 4], BF16, tag="c2bf")
        nc.vector.tensor_copy(out=c2bf, in_=c2)
        ptb = psC.tile([128, 1], F32, tag="av")
        for kc in range(4):
            nc.tensor.matmul(ptb, w2T[:, kc], c2bf[:, kc:kc + 1], start=(kc == 0), stop=(kc == 3))
        biasc = small.tile([128, 1], F32, tag="biascS")
        nc.vector.tensor_copy(out=biasc, in_=ptb)

        # pw2 -> h3s
        h3s = big.tile([128, BL, HW], F32, tag="big")
        h3f = h3s.rearrange("p b q -> p (b q)")
        st3 = small.tile([128, 1, NT, 6], F32, tag="bnst")
        mv3 = small.tile([128, 1, 2], F32, tag="bnmv")
        for i in range(NT):
            pt = psA.tile([128, TCH], F32, tag="mm")
            for kc in range(4):
                nc.tensor.matmul(pt, w2s[:, kc], h2f[:, kc, i * TCH:(i + 1) * TCH],
                                 start=(kc == 0), stop=(kc == 3))
            nc.vector.tensor_scalar(out=h3f[:, i * TCH:(i + 1) * TCH], in0=pt, scalar1=biasc,
                                    scalar2=None, op0=ALU.add)
            nc.vector.bn_stats(out=st3[:, 0, i], in_=h3f[:, i * TCH:(i + 1) * TCH])
        nc.vector.bn_aggr(out=mv3[:, 0], in_=st3[:, 0])
        stats3 = small.tile([128, 1, 2], F32, tag="bnpk3")
        m3b = small.tile([128, 1], F32, tag="bnm3")
        nc.vector.tensor_scalar(out=stats3[:, :, 0:1], in0=mv3[:, :, 0:1],
                                scalar1=float(T), scalar2=None, op0=ALU.mult)
        nc.vector.tensor_mul(out=m3b, in0=mv3[:, :, 0], in1=mv3[:, :, 1 - 1])
        nc.vector.tensor_add(out=m3b, in0=m3b, in1=mv3[:, :, 1])
        nc.vector.tensor_scalar(out=stats3[:, :, 1:2], in0=m3b.rearrange("p (e o) -> p e o", o=1),
                                scalar1=float(T), scalar2=None, op0=ALU.mult)
        nc.gpsimd.dma_start(out=ar_in[3][:], in_=stats3.rearrange("p e two -> p (e two)"))
        nc.gpsimd.collective_compute("AllReduce", ALU.add, RG,
                                     ins=[ar_in[3][:]], outs=[ar_out[3][:]])
        g3 = small.tile([128, 1, 2], F32, tag="bngl3")
        nc.gpsimd.dma_start(out=g3.rearrange("p e two -> p (e two)"), in_=ar_out[3][:])
        a3, c3 = bn_affine(g3, 1, bn2_g, bn2_b)

        nc.vector.tensor_scalar(out=h3f, in0=h3f, scalar1=a3, scalar2=c3,
                                op0=ALU.mult, op1=ALU.add)
        nc.vector.tensor_add(out=x_mhsa, in0=x_mhsa, in1=h3s)
        nc.sync.dma_start(out=out_t[:].rearrange("b c h w -> c b (h w)"), in_=x_mhsa)


_cached = None


def kernel(**inputs):
    global last_result, _cached
    hp = _host_prep(inputs)
    ln_triv = hp.pop("_ln_triv")
    if _cached is None or _cached[1] != ln_triv:
        _cached = (_build(ln_triv), ln_triv)
    nc = _cached[0]
    x = np.ascontiguousarray(np.asarray(inputs["x"], dtype=np.float32))
    in_maps = []
    for c in range(NC):
        m = dict(hp)
        m["xs"] = np.ascontiguousarray(x[c * BL:(c + 1) * BL])
        in_maps.append(m)
    trace = os.environ.get("KERNEL_TRACE", "0") == "1"
    res = run_bass_kernel_spmd(nc, in_maps, core_ids=list(range(NC)), trace=trace)
    last_result = res
    return np.concatenate([r["out"] for r in res.results], axis=0)



# revision 39
# speedup vs baseline: 1.0487x; 1.0487x over previous
import os, sys
import numpy as np

sys.path.insert(0, "/opt/trn_rl_repo")

import concourse.bass as bass
import concourse.bacc as bacc
import concourse.tile as tile
import concourse.mybir as mybir
from concourse.bass_utils import run_bass_kernel_spmd

F32 = mybir.dt.float32
F32R = mybir.dt.float32r
BF16 = mybir.dt.bfloat16
F8 = mybir.dt.float8e4
AF = mybir.ActivationFunctionType
ALU = mybir.AluOpType

NC = 8
B, C, H, W = 64, 128, 28, 28
BL = B // NC
HW = H * W
T = BL * HW                  # 6272
HEADS, D = 4, 32
E = 512
KV, L = 15, 225
EPS = 1e-5
NG = float(B * HW)
SCALE = D ** -0.5
NT, TCH = 14, 448
KC0, KC1 = 128, L - 128

last_result = None


def _f32r(ap):
    return ap.bitcast(F32R)


def _class_ranges(k):
    if k == 0:
        return (1, 2)
    if k == 1:
        return (0, 1, 2)
    return (0, 1)


def _host_prep(inputs):
    import ml_dtypes
    bf = ml_dtypes.bfloat16
    f = lambda a: np.ascontiguousarray(np.asarray(a), dtype=np.float32)
    inp = {k: np.asarray(v) for k, v in inputs.items()}
    h = {}

    def diag(wk, ntap, dt):
        ch = wk.shape[0]
        nch = ch // 128
        out = np.zeros((128, nch, ntap, 128), dtype=np.float32)
        for cc in range(nch):
            for t in range(ntap):
                out[np.arange(128), cc, t, np.arange(128)] = wk[cc * 128:(cc + 1) * 128, t]
        return np.ascontiguousarray(out.astype(dt))

    import ml_dtypes as mld0
    f8t = mld0.float8_e4m3

    def diag_pairs(wk, pairs, dt, scale=16.0):
        # wk [ch, ntap]; returns [128, nch, npair, 2, 128]
        ch = wk.shape[0]
        nch = ch // 128
        out = np.zeros((128, nch, len(pairs), 2, 128), dtype=np.float32)
        for cc in range(nch):
            for pi, (ta, tb) in enumerate(pairs):
                out[np.arange(128), cc, pi, 0, np.arange(128)] = wk[cc * 128:(cc + 1) * 128, ta] * scale
                out[np.arange(128), cc, pi, 1, np.arange(128)] = wk[cc * 128:(cc + 1) * 128, tb] * scale
        return np.ascontiguousarray(out.astype(dt))

    # 3x3 taps indexed kh*3+kw; pairs chosen with constant in-pad stride:
    # (0,0)+(0,2)->S2, (2,0)+(2,2)->S2, (0,1)+(2,1)->S60, (1,0)+(1,2)->S2
    P9 = [(0, 2), (6, 8), (1, 7), (3, 5)]
    lpu9 = f(inp["lpu_w"]).reshape(C, 9)
    h["lpur"] = diag_pairs(lpu9, P9, f8t).reshape(128, 4, 2, 128)
    h["lpuc"] = diag((lpu9[:, 4:5] * 16.0), 1, f8t).reshape(128, 128)
    # 2x2 stride-2 taps kh*2+kw; pairs (0,0)+(0,1)->S1, (1,0)+(1,1)->S1
    P4 = [(0, 1), (2, 3)]
    h["kdwr"] = diag_pairs(f(inp["kdw_w"]).reshape(C, 4), P4, f8t).reshape(128, 2, 2, 128)
    h["vdwr"] = diag_pairs(f(inp["vdw_w"]).reshape(C, 4), P4, f8t).reshape(128, 2, 2, 128)
    h["wqT"] = f(inp["wq"]).T.copy().astype(bf)
    h["wkT"] = f(inp["wk"]).T.copy().astype(bf)
    h["wvT"] = f(inp["wv"]).T.copy().astype(bf)
    h["woT"] = f(inp["wo"]).T.copy().astype(bf)
    h["bq"] = f(inp["bq"]).reshape(C, 1)
    h["bkp"] = (f(inp["bk"]) + f(inp["wk"]) @ f(inp["kdw_b"])).reshape(C, 1)
    bvp = f(inp["bv"]) + f(inp["wv"]) @ f(inp["vdw_b"])
    h["bop"] = (f(inp["bo"]) + f(inp["wo"]) @ bvp + f(inp["lpu_b"])).reshape(C, 1)
    import ml_dtypes as mld
    f8 = mld.float8_e4m3
    # raw attention bias (divided by softmax scale), keys-major, for PSUM
    # preload ahead of the QK matmul: et = exp(SCALE*(qk + bias/SCALE))
    bq_ = f(inp["attn_bias"])[0].transpose(0, 2, 1) / (D ** -0.5)  # [4, 225, 784]
    bqp = np.zeros((128, 2, HEADS, HW), dtype=np.float32)
    bqp[:, 0] = bq_[:, 0:128, :].transpose(1, 0, 2)
    bqp[:KC1, 1] = bq_[:, 128:L, :].transpose(1, 0, 2)
    h["biasq"] = np.ascontiguousarray(bqp.astype(f8))
    eb = np.exp(f(inp["attn_bias"]))[0].transpose(0, 2, 1)  # [4, 225, 784]
    ebp = np.zeros((128, 2, HEADS, HW), dtype=np.float32)
    ebp[:, 0] = eb[:, 0:128, :].transpose(1, 0, 2)
    ebp[:KC1, 1] = eb[:, 128:L, :].transpose(1, 0, 2)
    h["expb"] = np.ascontiguousarray(ebp.astype(f8))
    eb = np.exp(f(inp["attn_bias"]))[0].transpose(0, 2, 1)  # [4, 225, 784]
    ebp = np.zeros((128, 2, HEADS, HW), dtype=np.float32)
    ebp[:, 0] = eb[:, 0:128, :].transpose(1, 0, 2)
    ebp[:KC1, 1] = eb[:, 128:L, :].transpose(1, 0, 2)
    h["expb"] = np.ascontiguousarray(ebp.astype(f8))
    idp = np.zeros((128, 128), dtype=np.float32)
    idp[np.arange(128), np.arange(128)] = 1.0
    h["identp"] = np.ascontiguousarray(idp.astype(f8))
    dww = f(inp["dw_w"]).reshape(E, 3, 3).copy()
    dww[:, 1, 1] += 1.0
    dwr9 = f(inp["dw_w"]).reshape(E, 9)
    h["fdr"] = diag_pairs(dwr9, P9, f8t)                 # [128, 4, 4, 2, 128]
    h["fctr"] = diag((dwr9[:, 4:5] + 1.0) * 16.0, 1, bf).reshape(128, 4, 128)
    h["dw_b"] = f(inp["dw_b"]).reshape(4, 128).T.copy()
    psum9 = np.zeros((9, 4, 128), dtype=np.float32)
    for k in range(9):
        hr, wr = _class_ranges(k // 3), _class_ranges(k % 3)
        s = dww[:, hr, :][:, :, wr].sum(axis=(1, 2))
        psum9[k] = s.reshape(4, 128)
    h["psum9"] = psum9 * 16.0
    ind9 = np.zeros((9, H, W), dtype=np.float32)
    hc = np.full(H, 1); hc[0] = 0; hc[-1] = 2
    wc = np.full(W, 1); wc[0] = 0; wc[-1] = 2
    for i in range(H):
        for j in range(W):
            ind9[hc[i] * 3 + wc[j], i, j] = 1.0
    h["ind9"] = ind9.reshape(9, HW).astype(bf)
    h["c1wT"] = f(inp["c1_w"]).T.copy().astype(bf)
    h["c1_b"] = f(inp["c1_b"]).reshape(4, 128).T.copy()
    h["w2T"] = f(inp["c2_w"]).T.reshape(4, 128, 128).transpose(1, 0, 2).copy().astype(bf)
    h["bn1_g"] = f(inp["bn1_g"]).reshape(4, 128).T.copy()
    h["bn1_b"] = f(inp["bn1_b"]).reshape(4, 128).T.copy()
    h["bnr_g"] = f(inp["bnr_g"]).reshape(4, 128).T.copy()
    h["bnr_b"] = f(inp["bnr_b"]).reshape(4, 128).T.copy()
    h["bn2_g"] = f(inp["bn2_g"]).reshape(C, 1)
    h["bn2_b"] = f(inp["bn2_b"]).reshape(C, 1)
    ln_triv = (np.allclose(inp["ln1_g"], 1) and np.allclose(inp["ln1_b"], 0)
               and np.allclose(inp["ln2_g"], 1) and np.allclose(inp["ln2_b"], 0))
    h["_ln_triv"] = ln_triv
    if not ln_triv:
        h["ln1_g"] = f(inp["ln1_g"]).reshape(1, HW)
        h["ln1_b"] = f(inp["ln1_b"]).reshape(1, HW)
        h["ln2_g"] = f(inp["ln2_g"]).reshape(1, HW)
        h["ln2_b"] = f(inp["ln2_b"]).reshape(1, HW)
    return h


def _build(ln_triv):
    nc = bacc.Bacc(None, target_bir_lowering=False, num_devices=NC)
    dt = nc.dram_tensor
    xs = dt("xs", [BL, C, H, W], F32, kind="ExternalInput")
    out_t = dt("out", [BL, C, H, W], F32, kind="ExternalOutput")
    hin = {}
    specs = [
        ("lpur", [128, 4, 2, 128], F8), ("lpuc", [128, 128], F8),
        ("kdwr", [128, 2, 2, 128], F8), ("vdwr", [128, 2, 2, 128], F8),
        ("fdr", [128, 4, 4, 2, 128], F8), ("fctr", [128, 4, 128], BF16),
        ("wqT", [C, C], BF16), ("wkT", [C, C], BF16), ("wvT", [C, C], BF16),
        ("woT", [C, C], BF16), ("bq", [C, 1], F32), ("bkp", [C, 1], F32),
        ("bop", [C, 1], F32), ("expb", [128, 2, HEADS, HW], F8),
        ("dw_b", [128, 4], F32),
        ("psum9", [9, 4, 128], F32), ("ind9", [9, HW], BF16),
        ("c1wT", [C, E], BF16), ("c1_b", [128, 4], F32),
        ("w2T", [128, 4, 128], BF16),
        ("bn1_g", [128, 4], F32), ("bn1_b", [128, 4], F32),
        ("bnr_g", [128, 4], F32), ("bnr_b", [128, 4], F32),
        ("bn2_g", [C, 1], F32), ("bn2_b", [C, 1], F32),
    ]
    if not ln_triv:
        specs += [(n, [1, HW], F32) for n in ["ln1_g", "ln1_b", "ln2_g", "ln2_b"]]
    for name, shape, d in specs:
        hin[name] = dt(name, shape, d, kind="ExternalInput")
    ar_in = {0: dt("ar0i", [128, 1], F32, kind="Internal"),
             1: dt("ar1i", [128, 8], F32, kind="Internal"),
             2: dt("ar2i", [128, 8], F32, kind="Internal"),
             3: dt("ar3i", [128, 2], F32, kind="Internal")}
    ar_out = {0: dt("ar0o", [128, 1], F32, kind="Internal", addr_space="Shared"),
              1: dt("ar1o", [128, 8], F32, kind="Internal", addr_space="Shared"),
              2: dt("ar2o", [128, 8], F32, kind="Internal", addr_space="Shared"),
              3: dt("ar3o", [128, 2], F32, kind="Internal", addr_space="Shared")}
    c1_dram = dt("c1d", [128, 4], F32, kind="Internal")
    RG = [list(range(NC))]
    with tile.TileContext(nc) as tc:
        _emit(nc, tc, xs, out_t, hin, ar_in, ar_out, c1_dram, RG, ln_triv)
    if not nc.is_finalized():
        nc.finalize()
    return nc


def _emit(nc, tc, xs, out_t, hin, ar_in, ar_out, c1_dram, RG, ln_triv):
    from contextlib import ExitStack
    ctx = ExitStack()
    with ctx:
        big = ctx.enter_context(tc.tile_pool(name="big", bufs=2))
        bfp = ctx.enter_context(tc.tile_pool(name="bfp", bufs=1))
        cons = ctx.enter_context(tc.tile_pool(name="cons", bufs=1))
        small = ctx.enter_context(tc.tile_pool(name="small", bufs=1))
        etp = ctx.enter_context(tc.tile_pool(name="etp", bufs=8))
        psA = ctx.enter_context(tc.tile_pool(name="psA", bufs=4, space="PSUM"))
        psB = ctx.enter_context(tc.tile_pool(name="psB", bufs=2, space="PSUM"))
        psC = ctx.enter_context(tc.tile_pool(name="psC", bufs=2, space="PSUM"))

        def loadc(name):
            hh = hin[name]
            t = cons.tile(list(hh.shape), hh.dtype, tag=name)
            nc.gpsimd.dma_start(out=t, in_=hh[:])
            return t

        nc.gpsimd.collective_compute("AllReduce", ALU.add, RG,
                                     ins=[ar_in[0][:]], outs=[ar_out[0][:]])
        lpur = loadc("lpur"); lpuc = loadc("lpuc")
        kdwr = loadc("kdwr"); vdwr = loadc("vdwr")
        fdr = loadc("fdr"); fctr = loadc("fctr")
        wqT = loadc("wqT"); wkT = loadc("wkT"); wvT = loadc("wvT"); woT = loadc("woT")
        bq = loadc("bq"); bkp = loadc("bkp"); bop = loadc("bop")
        expbt = loadc("expb")
        dw_b = loadc("dw_b")
        psum9 = loadc("psum9"); ind9t = loadc("ind9")
        c1wT = loadc("c1wT"); c1_b = loadc("c1_b"); w2T = loadc("w2T")
        bn1_g = loadc("bn1_g"); bn1_b = loadc("bn1_b")
        bnr_g = loadc("bnr_g"); bnr_b = loadc("bnr_b")
        bn2_g = loadc("bn2_g"); bn2_b = loadc("bn2_b")
        ind9 = ind9t.rearrange("k (h w) -> k h w", h=H)
        lns = {}
        if not ln_triv:
            for nm in ["ln1_g", "ln1_b", "ln2_g", "ln2_b"]:
                t = cons.tile([128, HW], F32, tag=nm)
                nc.gpsimd.dma_start(out=t, in_=bass.AP(tensor=hin[nm], offset=0, ap=[[0, 128], [1, HW]]))
                lns[nm] = t
        epsT = small.tile([128, 1], F32, tag="epsT")
        nc.vector.memset(epsT, EPS)
        # pre-touch DMA-loaded consts on the engines that read them, so heavy
        # ops don't accumulate multiple DMA-queue sem waits (codegen limit)
        scrD = small.tile([128, 1], F32, tag="scrD")
        scrA = small.tile([128, 1], F32, tag="scrA")
        for t2 in (bq, bkp, bop, bn2_g, bn2_b):
            nc.vector.tensor_copy(out=scrD, in_=t2[:, 0:1])
        nc.vector.tensor_copy(out=scrD, in_=lpur[:, 0, 0, 0:1])
        nc.vector.tensor_copy(out=scrD, in_=kdwr[:, 0, 0, 0:1])
        nc.vector.tensor_copy(out=scrD, in_=vdwr[:, 0, 0, 0:1])
        nc.vector.tensor_copy(out=scrD, in_=w2T[:, 0, 0:1])
        nc.vector.tensor_copy(out=scrD, in_=lpuc[:, 0:1])
        nc.vector.tensor_copy(out=scrD, in_=fdr[:, 0, 0, 0, 0:1])
        nc.vector.tensor_copy(out=scrD, in_=fctr[:, 0, 0:1])
        for t4 in (wqT, wkT, wvT, woT, c1wT):
            nc.vector.tensor_copy(out=scrD, in_=t4[:, 0:1])
        nc.vector.tensor_copy(out=scrD, in_=expbt[:, 0, 0, 0:1])
        for t5 in (dw_b, c1_b, bn1_g, bn1_b, bnr_g, bnr_b):
            nc.vector.tensor_copy(out=scrD, in_=t5[:, 0:1])
        nc.vector.tensor_copy(out=scrD[0:9], in_=psum9[:, 0, 0:1])
        nc.vector.tensor_copy(out=scrD[0:9], in_=ind9t[:, 0:1])
        for t6 in lns.values():
            nc.vector.tensor_copy(out=scrD, in_=t6[:, 0:1])
        nc.scalar.mul(out=scrA, in_=c1_b[:, 0:1], mul=1.0)
        nc.scalar.mul(out=scrA, in_=dw_b[:, 0:1], mul=1.0)

        xsb = big.tile([128, BL, HW], F32, tag="big")
        nc.gpsimd.dma_start(out=xsb, in_=xs[:].rearrange("b c h w -> c b (h w)"))
        nc.vector.tensor_copy(out=scrD, in_=xsb[:, 0, 0:1])
        # zero-padded fp8 copy of x: [128, BL, 30, 30], image at [1:29, 1:29]
        xpad = bfp.tile([128, BL, 30, 30], F8, tag="pad8")
        nc.vector.memset(xpad[:, :, 0, :], 0.0)
        nc.vector.memset(xpad[:, :, 29, :], 0.0)
        nc.vector.memset(xpad[:, :, 1:29, 0], 0.0)
        nc.vector.memset(xpad[:, :, 1:29, 29], 0.0)
        with nc.allow_low_precision("conv input in fp8"):
            xsb4v = xsb.rearrange("p b (h w) -> p b h w", h=H)
            for b in range(BL):
                nc.gpsimd.tensor_copy(out=xpad[:, b, 1:29, 1:29], in_=xsb4v[:, b])

        # x16-scaled fp8 DoubleRow taps; psum holds 16*dw(x); the +x residual
        # and /16 happen at evac.  Pair t reads (khA,kwA)/(khB,kwB) windows of
        # the padded image via an overlapping stride-S access pattern.
        P9T = [((0, 0), (0, 2)), ((2, 0), (2, 2)), ((0, 1), (2, 1)), ((1, 0), (1, 2))]

        def pad_pair_rhs(padt, pre, b, base, pa, pb_):
            (ka, wa), (kb, wb) = pa, pb_
            S = (kb - ka) * 30 + (wb - wa)
            off = padt.offset + (pre + b) * 900 + (base + ka) * 30 + wa
            return bass.AP(tensor=padt.tensor, offset=off,
                           ap=[list(padt.ap[0]), [S, 2], [30, 14], [1, 28]])

        def pad_tap_rhs(padt, pre, b, base, kh, kw):
            off = padt.offset + (pre + b) * 900 + (base + kh) * 30 + kw
            return bass.AP(tensor=padt.tensor, offset=off,
                           ap=[list(padt.ap[0]), [30, 14], [1, 28]])

        x_lpu = big.tile([128, BL, HW], F32, tag="big")
        xlp4 = x_lpu.rearrange("p b (h w) -> p b h w", h=H)
        xsb4 = xsb.rearrange("p b (h w) -> p b h w", h=H)

        for half in range(2):
            base = 14 * half
            pts = []
            for b in range(BL):
                pool = (psA, psA, psA, psA, psB, psB, psC, psC)[b]
                tg = ("mm", "mm", "mm", "mm", "dwp", "dwp", "av", "av")[b]
                pts.append(pool.tile([128, 14, W], F32, tag=tg, name=f"lvp{b}"))
            for pi in range(4):
                pa, pb_ = P9T[pi]
                for b in range(BL):
                    nc.tensor.matmul(
                        pts[b], lpur[:, pi], pad_pair_rhs(xpad, 0, b, base, pa, pb_),
                        start=(pi == 0), stop=False,
                        perf_mode=mybir.MatmulPerfMode.DoubleRow,
                        skip_group_check=True)
            for b in range(BL):
                nc.tensor.matmul(
                    pts[b], lpuc, pad_tap_rhs(xpad, 0, b, base, 1, 1),
                    start=False, stop=True, skip_group_check=True)
            for b in range(BL):
                nc.vector.scalar_tensor_tensor(
                    out=xlp4[:, b, base:base + 14, :], in0=pts[b], scalar=1.0 / 16.0,
                    in1=xsb4[:, b, base:base + 14, :], op0=ALU.mult, op1=ALU.add)

        # LN over HW
        def layer_norm(src, gname, dst):
            sv = src.rearrange("p b (two q) -> p b two q", two=2)
            st = small.tile([128, BL, 2, 6], F32, tag="lnst")
            mv = small.tile([128, BL, 2], F32, tag="lnmv")
            sd = small.tile([128, BL, 1], F32, tag="lnsd")
            for b in range(BL):
                for g2 in range(2):
                    nc.vector.bn_stats(out=st[:, b, g2], in_=sv[:, b, g2])
                nc.vector.bn_aggr(out=mv[:, b], in_=st[:, b])
            nc.scalar.activation(out=sd, in_=mv[:, :, 1:2], func=AF.Sqrt, bias=epsT, scale=1.0)
            nc.vector.reciprocal(out=sd, in_=sd)
            for b in range(BL):
                nc.vector.tensor_scalar(
                    out=dst[:, b], in0=src[:, b], scalar1=mv[:, b, 0:1], scalar2=sd[:, b],
                    op0=ALU.subtract, op1=ALU.mult)
            if not ln_triv:
                g = lns[gname + "_g"]; bb = lns[gname + "_b"]
                for b in range(BL):
                    nc.vector.tensor_mul(out=dst[:, b], in0=dst[:, b], in1=g)
                    nc.vector.tensor_add(out=dst[:, b], in0=dst[:, b], in1=bb)

        xnbf = bfp.tile([128, BL, HW], BF16, tag="t12b")
        layer_norm(x_lpu, "ln1", xnbf)
        xnpad = bfp.tile([128, BL, 30, 30], F8, tag="pad8")
        nc.vector.memset(xnpad[:, :, 0, :], 0.0)
        nc.vector.memset(xnpad[:, :, 29, :], 0.0)
        nc.vector.memset(xnpad[:, :, 1:29, 0], 0.0)
        nc.vector.memset(xnpad[:, :, 1:29, 29], 0.0)
        with nc.allow_low_precision("conv input in fp8"):
            xnbf4v = xnbf.rearrange("p b (h w) -> p b h w", h=H)
            for b in range(BL):
                nc.gpsimd.tensor_copy(out=xnpad[:, b, 1:29, 1:29], in_=xnbf4v[:, b])

        # Q projection (f32r) -> bf16
        qbf = bfp.tile([128, BL, HW], BF16, tag="qbf")
        xnbff = xnbf.rearrange("p b q -> p (b q)")
        qbff = qbf.rearrange("p b q -> p (b q)")
        for i in range(NT):
            pt = psA.tile([128, TCH], F32, tag="mm")
            nc.tensor.matmul(pt, wqT, xnbff[:, i * TCH:(i + 1) * TCH], start=True, stop=True)
            nc.vector.tensor_scalar(out=qbff[:, i * TCH:(i + 1) * TCH], in0=pt, scalar1=bq,
                                    scalar2=None, op0=ALU.add)
        # K/V strided 2x2 dw conv
        kxbf = bfp.tile([128, BL, L], BF16, tag="kxbf")
        vxbf = bfp.tile([128, BL, L], BF16, tag="vxbf")
        kx4 = kxbf.rearrange("p b (i j) -> p b i j", i=KV)
        vx4 = vxbf.rearrange("p b (i j) -> p b i j", i=KV)
        # kv conv: out(i,j) = sum w[kh,kw]*xn[2i+kh-1, 2j+kw-1]
        #        = sum w[kh,kw]*xnpad[2i+kh, 2j+kw], i,j in [0,15)
        def kv_pair_rhs(b, kh):
            off = xnpad.offset + b * 900 + kh * 30
            return bass.AP(tensor=xnpad.tensor, offset=off,
                           ap=[list(xnpad.ap[0]), [1, 2], [60, KV], [2, KV]])

        for b in range(BL):
            for dst4, dg in ((kx4, kdwr), (vx4, vdwr)):
                pt = psA.tile([128, KV, KV], F32, tag="mm")
                for kh in range(2):
                    nc.tensor.matmul(
                        pt, dg[:, kh], kv_pair_rhs(b, kh),
                        start=(kh == 0), stop=(kh == 1),
                        perf_mode=mybir.MatmulPerfMode.DoubleRow,
                        skip_group_check=True)
                nc.scalar.activation(out=dst4[:, b], in_=pt, func=AF.Copy,
                                     scale=1.0 / 16.0)
        kbf = bfp.tile([128, BL, L], BF16, tag="kbf")
        kxf = kxbf.rearrange("p b l -> p (b l)")
        kbff = kbf.rearrange("p b l -> p (b l)")
        for i in range(4):
            pt = psA.tile([128, 450], F32, tag="mm")
            nc.tensor.matmul(pt, wkT, kxf[:, i * 450:(i + 1) * 450], start=True, stop=True)
            nc.vector.tensor_scalar(out=kbff[:, i * 450:(i + 1) * 450], in0=pt, scalar1=bkp,
                                    scalar2=None, op0=ALU.add)
        vaug = bfp.tile([128, BL, 2, HEADS, 64], F8, tag="vaug")
        nc.vector.memset(vaug, 0.0)
        nc.vector.memset(vaug[:, :, :, :, 32:64], 1.0)
        for b in range(BL):
            for kc in range(2):
                ktM = KC0 if kc == 0 else KC1
                pt = psA.tile([128, 128], F32, tag="mm")
                nc.tensor.matmul(pt[0:ktM], vxbf[:, b, kc * 128: kc * 128 + ktM], wvT,
                                 start=True, stop=True)
                with nc.allow_low_precision("attention V in fp8"):
                    nc.scalar.copy(out=vaug[0:ktM, b, kc, :, 0:32],
                                   in_=pt[0:ktM].rearrange("p (h d) -> p h d", h=HEADS))

        # attention: QK into psum (4 heads row-tiled), then the attention bias
        # preloaded on top via one full identity matmul; exp at evac includes
        # the bias; AV with a ones-block for denominators; fast-approx
        # reciprocal of the denominators.
        o_sb = bfp.tile([128, BL, HW], BF16, tag="t12a")
        rbc = bfp.tile([128, BL, HW], F32, tag="h1h2")
        for b in range(BL):
            dscr = bfp.tile([128, HW], F32, tag="kxbf", name=f"dscr{b}")
            for qc in range(2):
                qs = slice(qc * 392, (qc + 1) * 392)
                ets = {}
                for kc in range(2):
                    ktM = KC0 if kc == 0 else KC1
                    pts = []
                    for hd in range(HEADS):
                        pt = psA.tile([128, 392], F32, tag="mm")
                        nc.tensor.matmul(
                            pt[0:ktM],
                            kbf[hd * 32:(hd + 1) * 32, b, kc * 128: kc * 128 + ktM],
                            qbf[hd * 32:(hd + 1) * 32, b, qs],
                            start=True, stop=True, tile_position=(hd * 32, 0))
                        pts.append(pt)
                    for hd in range(HEADS):
                        pt = pts[hd]
                        et = etp.tile([128, 392], F8, tag="et")
                        with nc.allow_low_precision("attention scores fp8"):
                            nc.scalar.activation(out=et[0:ktM], in_=pt[0:ktM],
                                                 func=AF.Exp, scale=SCALE)
                            nc.vector.tensor_mul(out=et[0:ktM], in0=et[0:ktM],
                                                 in1=expbt[0:ktM, kc, hd, qs])
                        ets[(kc, hd)] = et
                for hp in range(2):
                    pv = psC.tile([128, 392], F32, tag="av")
                    for kc in range(2):
                        ktM = KC0 if kc == 0 else KC1
                        for hh in range(2):
                            hd = hp * 2 + hh
                            nc.tensor.matmul(
                                pv[64 * hh:64 * hh + 64],
                                vaug[0:ktM, b, kc, hd, :], ets[(kc, hd)][0:ktM],
                                start=(kc == 0), stop=(kc == 1),
                                tile_position=(0, 64 * hh), skip_group_check=True)
                    for hh in range(2):
                        hd = hp * 2 + hh
                        with nc.allow_low_precision("attention numerators bf16"):
                            nc.vector.tensor_copy(
                                out=o_sb[hd * 32:(hd + 1) * 32, b, qs],
                                in_=pv[64 * hh:64 * hh + 32])
                        nc.scalar.copy(
                            out=dscr[hd * 32:(hd + 1) * 32, qs],
                            in_=pv[64 * hh + 32:64 * hh + 64])
                # fast reciprocal of this (b, qc)'s denominators (packed dup
                # layout): magic seed + one Newton, standard ops.  rbc ends
                # NEGATED ((x*y0-2)*y0 = -1/x); the o-mul flips the sign.
                nc.vector.tensor_scalar(
                    out=rbc[:, b, qs].bitcast(mybir.dt.int32),
                    in0=dscr[:, qs].bitcast(mybir.dt.int32),
                    scalar1=-1, scalar2=0x7EF127EA, op0=ALU.mult, op1=ALU.add)
                nc.vector.tensor_mul(out=dscr[:, qs], in0=dscr[:, qs], in1=rbc[:, b, qs])
                nc.vector.scalar_tensor_tensor(
                    out=rbc[:, b, qs], in0=dscr[:, qs], scalar=2.0, in1=rbc[:, b, qs],
                    op0=ALU.subtract, op1=ALU.mult)
        nc.vector.scalar_tensor_tensor(out=o_sb, in0=o_sb, scalar=-1.0, in1=rbc,
                                       op0=ALU.mult, op1=ALU.mult)

        x_mhsa = big.tile([128, BL, HW], F32, tag="big")
        of = o_sb.rearrange("p b q -> p (b q)")
        xmf = x_mhsa.rearrange("p b q -> p (b q)")
        xlf = x_lpu.rearrange("p b q -> p (b q)")
        for i in range(NT):
            pt = psA.tile([128, TCH], F32, tag="mm")
            nc.tensor.matmul(pt, woT, of[:, i * TCH:(i + 1) * TCH], start=True, stop=True)
            nc.vector.scalar_tensor_tensor(out=xmf[:, i * TCH:(i + 1) * TCH], in0=pt, scalar=bop,
                                           in1=xlf[:, i * TCH:(i + 1) * TCH], op0=ALU.add, op1=ALU.add)

        ybf = bfp.tile([128, BL, HW], BF16, tag="t12a")
        layer_norm(x_mhsa, "ln2", ybf)

        def bn_reduce(src_r, nchunk, ar_i, ar_o):
            # global batch statistics: local bn_stats/aggr, then a cross-core
            # AllReduce of (mean*T, (mean^2+var)*T)
            st = small.tile([128, nchunk, NT, 6], F32, tag="bnst")
            mv = small.tile([128, nchunk, 2], F32, tag="bnmv")
            for ecx in range(nchunk):
                for i in range(NT):
                    nc.vector.bn_stats(out=st[:, ecx, i], in_=src_r[:, ecx, i])
                nc.vector.bn_aggr(out=mv[:, ecx], in_=st[:, ecx])
            stats = small.tile([128, nchunk, 2], F32, tag="bnpack")
            m2 = small.tile([128, nchunk], F32, tag="bnm2")
            nc.vector.tensor_scalar(out=stats[:, :, 0:1], in0=mv[:, :, 0:1], scalar1=float(T),
                                    scalar2=None, op0=ALU.mult)
            nc.vector.tensor_mul(out=m2, in0=mv[:, :, 0], in1=mv[:, :, 0])
            nc.vector.tensor_add(out=m2, in0=m2, in1=mv[:, :, 1])
            nc.vector.tensor_scalar(out=stats[:, :, 1:2], in0=m2.rearrange("p (e o) -> p e o", o=1),
                                    scalar1=float(T), scalar2=None, op0=ALU.mult)
            nc.gpsimd.dma_start(out=ar_i[:], in_=stats.rearrange("p e two -> p (e two)"))
            nc.gpsimd.collective_compute("AllReduce", ALU.add, RG, ins=[ar_i[:]], outs=[ar_o[:]])
            g = small.tile([128, nchunk, 2], F32, tag="bngl")
            nc.gpsimd.dma_start(out=g.rearrange("p e two -> p (e two)"), in_=ar_o[:])
            return g

        def bn_affine(gs, nchunk, gt, bt, want_coa=False):
            a = small.tile([128, nchunk], F32, tag="bna")
            cc = small.tile([128, nchunk], F32, tag="bnc")
            mean = small.tile([128, nchunk], F32, tag="bnmean")
            m2 = small.tile([128, nchunk], F32, tag="bnm2b")
            nc.vector.tensor_scalar(out=mean, in0=gs[:, :, 0], scalar1=1.0 / NG, scalar2=None, op0=ALU.mult)
            nc.vector.tensor_scalar(out=a, in0=gs[:, :, 1], scalar1=1.0 / NG, scalar2=None, op0=ALU.mult)
            nc.vector.tensor_mul(out=m2, in0=mean, in1=mean)
            nc.vector.tensor_sub(out=a, in0=a, in1=m2)
            nc.scalar.activation(out=a, in_=a, func=AF.Sqrt, bias=epsT, scale=1.0)
            nc.vector.reciprocal(out=a, in_=a)
            nc.vector.tensor_mul(out=a, in0=a, in1=gt)
            nc.vector.tensor_mul(out=cc, in0=mean, in1=a)
            nc.vector.scalar_tensor_tensor(out=cc, in0=cc, scalar=-1.0, in1=bt,
                                           op0=ALU.mult, op1=ALU.add)
            if not want_coa:
                return a, cc
            ra = small.tile([128, nchunk], F32, tag="bnra")
            coa = small.tile([128, nchunk], F32, tag="bncoa")
            nc.vector.reciprocal(out=ra, in_=a)
            nc.vector.tensor_mul(out=coa, in0=cc, in1=ra)
            return a, cc, coa

        # pw1 + gelu -> h1bf
        h1bf = bfp.tile([128, 4, BL, HW], BF16, tag="h1h2")
        h1f = h1bf.rearrange("p e b q -> p e (b q)")
        ybff = ybf.rearrange("p b q -> p (b q)")
        for ec in range(4):
            for i in range(NT):
                pt = psA.tile([128, TCH], F32, tag="mm")
                nc.tensor.matmul(pt, c1wT[:, ec * 128:(ec + 1) * 128],
                                 ybff[:, i * TCH:(i + 1) * TCH], start=True, stop=True)
                nc.scalar.activation(out=h1f[:, ec, i * TCH:(i + 1) * TCH], in_=pt, func=AF.Gelu,
                                     bias=c1_b[:, ec:ec + 1], scale=1.0)
        # BN1: local stats feed the border-correction weights (lh9) so the
        # conv pipeline never waits on a collective; the gelu-evac scale/bias
        # (a1, via a1o16) uses the exact global stats from the AllReduce.
        st1l = small.tile([128, 4, NT, 6], F32, tag="bnst")
        mv1l = small.tile([128, 4, 2], F32, tag="bnmv")
        h1r = h1f.rearrange("p e (n q) -> p e n q", q=TCH)
        for ecx in range(4):
            for i in range(NT):
                nc.vector.bn_stats(out=st1l[:, ecx, i], in_=h1r[:, ecx, i])
            nc.vector.bn_aggr(out=mv1l[:, ecx], in_=st1l[:, ecx])
        stats1 = small.tile([128, 4, 2], F32, tag="bnpack")
        m2l = small.tile([128, 4], F32, tag="bnm2l")
        nc.vector.tensor_scalar(out=stats1[:, :, 0:1], in0=mv1l[:, :, 0:1],
                                scalar1=float(T), scalar2=None, op0=ALU.mult)
        nc.vector.tensor_mul(out=m2l, in0=mv1l[:, :, 0], in1=mv1l[:, :, 0])
        nc.vector.tensor_add(out=m2l, in0=m2l, in1=mv1l[:, :, 1])
        nc.vector.tensor_scalar(out=stats1[:, :, 1:2], in0=m2l.rearrange("p (e o) -> p e o", o=1),
                                scalar1=float(T), scalar2=None, op0=ALU.mult)
        nc.gpsimd.dma_start(out=ar_in[1][:], in_=stats1.rearrange("p e two -> p (e two)"))
        nc.gpsimd.collective_compute("AllReduce", ALU.add, RG,
                                     ins=[ar_in[1][:]], outs=[ar_out[1][:]])
        gs1l = small.tile([128, 4, 2], F32, tag="bngl1")
        nc.vector.tensor_scalar(out=gs1l, in0=stats1, scalar1=float(NG / T),
                                scalar2=None, op0=ALU.mult)
        _a1l, _c1l, coa1 = bn_affine(gs1l, 4, bn1_g, bn1_b, want_coa=True)
        nc.gpsimd.dma_start(out=c1_dram[:], in_=coa1)
        c1row = small.tile([1, 4, 128], F32, tag="c1row")
        nc.gpsimd.dma_start(out=c1row, in_=bass.AP(tensor=c1_dram, offset=0, ap=[[0, 1], [1, 4], [4, 128]]))
        c1f = small.tile([9, 4, 128], F32, tag="c1f")
        nc.gpsimd.partition_broadcast(c1f, c1row)
        lh9 = small.tile([9, 4, 128], BF16, tag="lh9")
        nc.vector.tensor_mul(out=lh9, in0=psum9, in1=c1f)
        g1 = small.tile([128, 4, 2], F32, tag="bngl")
        nc.gpsimd.dma_start(out=g1.rearrange("p e two -> p (e two)"), in_=ar_out[1][:])
        a1, c1 = bn_affine(g1, 4, bn1_g, bn1_b)
        a1o16 = small.tile([128, 4], F32, tag="a1o16")
        nc.vector.tensor_scalar(out=a1o16, in0=a1, scalar1=1.0 / 16.0, scalar2=None,
                                op0=ALU.mult)

        # FFN dw via x16 fp8 pairs + fp8 center + bf16 identity (the +h1
        # residual) + border-count correction; BN1 scale a1 applied at evac.
        # The padded fp8 copy of h1 is built per-ec (ring of 2) to fit SBUF.
        h2g = h1bf
        h2g4 = h2g.rearrange("p e b (h w) -> p e b h w", h=H)
        h1b4 = h1bf.rearrange("p e b (h w) -> p e b h w", h=H)
        h2f = h2g.rearrange("p e b q -> p e (b q)")
        h2r = h2f.rearrange("p e (n q) -> p e n q", q=TCH)
        st2 = small.tile([128, 4, NT, 6], F32, tag="bnst")
        mv2 = small.tile([128, 4, 2], F32, tag="bnmv")
        for ec in range(4):
            gfp8 = bfp.tile([128, BL, 30, 30], F8, tag="gf8", bufs=2, name=f"gch{ec}")
            nc.vector.memset(gfp8[:, :, 0, :], 0.0)
            nc.vector.memset(gfp8[:, :, 29, :], 0.0)
            nc.vector.memset(gfp8[:, :, 1:29, 0], 0.0)
            nc.vector.memset(gfp8[:, :, 1:29, 29], 0.0)
            with nc.allow_low_precision("conv input in fp8"):
                nc.vector.tensor_copy(
                    out=gfp8[:, :, 1:29, 1:29],
                    in_=h1b4[:, ec])
            for half in range(2):
                base = 14 * half
                pts = []
                for b in range(BL):
                    pool = (psA, psA, psA, psA, psB, psB, psC, psC)[b]
                    tg = ("mm", "mm", "mm", "mm", "dwp", "dwp", "av", "av")[b]
                    pts.append(pool.tile([128, 14, W], F32, tag=tg, name=f"cvp{b}"))
                for pi in range(4):
                    pa, pb_ = P9T[pi]
                    for b in range(BL):
                        nc.tensor.matmul(
                            pts[b], fdr[:, ec, pi],
                            pad_pair_rhs(gfp8, 0, b, base, pa, pb_),
                            start=(pi == 0), stop=False,
                            perf_mode=mybir.MatmulPerfMode.DoubleRow,
                            skip_group_check=True)
                for b in range(BL):
                    nc.tensor.matmul(
                        pts[b], fctr[:, ec], h1b4[:, ec, b, base:base + 14, :],
                        start=False, stop=False, skip_group_check=True)
                for b in range(BL):
                    nc.tensor.matmul(pts[b], lh9[:, ec], ind9[:, base:base + 14, :],
                                     start=False, stop=True, skip_group_check=True)
                for b in range(BL):
                    nc.scalar.activation(out=h2g4[:, ec, b, base:base + 14, :], in_=pts[b],
                                         func=AF.Gelu, bias=dw_b[:, ec:ec + 1],
                                         scale=a1o16[:, ec:ec + 1])
                if half == 1:
                    # per-ec stats as soon as this ec's evacs land, so only
                    # the AllReduce tail is exposed after the conv
                    for i in range(NT):
                        nc.vector.bn_stats(out=st2[:, ec, i], in_=h2r[:, ec, i])
                    nc.vector.bn_aggr(out=mv2[:, ec], in_=st2[:, ec])
        stats2 = small.tile([128, 4, 2], F32, tag="bnpack")
        m2b = small.tile([128, 4], F32, tag="bnm2")
        nc.vector.tensor_scalar(out=stats2[:, :, 0:1], in0=mv2[:, :, 0:1],
                                scalar1=float(T), scalar2=None, op0=ALU.mult)
        nc.vector.tensor_mul(out=m2b, in0=mv2[:, :, 0], in1=mv2[:, :, 0])
        nc.vector.tensor_add(out=m2b, in0=m2b, in1=mv2[:, :, 1])
        nc.vector.tensor_scalar(out=stats2[:, :, 1:2], in0=m2b.rearrange("p (e o) -> p e o", o=1),
                                scalar1=float(T), scalar2=None, op0=ALU.mult)
        nc.gpsimd.dma_start(out=ar_in[2][:], in_=stats2.rearrange("p e two -> p (e two)"))
        nc.gpsimd.collective_compute("AllReduce", ALU.add, RG,
                                     ins=[ar_in[2][:]], outs=[ar_out[2][:]])
        g2 = small.tile([128, 4, 2], F32, tag="bngl")
        nc.gpsimd.dma_start(out=g2.rearrange("p e two -> p (e two)"), in_=ar_out[2][:])
        a2, c2 = bn_affine(g2, 4, bnr_g, bnr_b)
        w2s = bfp.tile([128, 4, 128], BF16, tag="t12a")
        for kc in range(4):
            nc.vector.tensor_scalar(out=w2s[:, kc], in0=w2T[:, kc], scalar1=a2[:, kc:kc + 1],
                                    scalar2=None, op0=ALU.mult)
        c2bf = small.tile([128, 4], BF16, tag="c2bf")
        nc.vector.tensor_copy(out=c2bf, in_=c2)
        ptb = psC.tile([128, 1], F32, tag="av")
        for kc in range(4):
            nc.tensor.matmul(ptb, w2T[:, kc], c2bf[:, kc:kc + 1], start=(kc == 0), stop=(kc == 3))
        biasc = small.tile([128, 1], F32, tag="biascS")
        nc.vector.tensor_copy(out=biasc, in_=ptb)

        # pw2 -> h3s
        h3s = big.tile([128, BL, HW], F32, tag="big")
        h3f = h3s.rearrange("p b q -> p (b q)")
        st3 = small.tile([128, 1, NT, 6], F32, tag="bnst")
        mv3 = small.tile([128, 1, 2], F32, tag="bnmv")
        for i in range(NT):
            pt = psA.tile([128, TCH], F32, tag="mm")
            for kc in range(4):
                nc.tensor.matmul(pt, w2s[:, kc], h2f[:, kc, i * TCH:(i + 1) * TCH],
                                 start=(kc == 0), stop=(kc == 3))
            nc.vector.tensor_scalar(out=h3f[:, i * TCH:(i + 1) * TCH], in0=pt, scalar1=biasc,
                                    scalar2=None, op0=ALU.add)
            nc.vector.bn_stats(out=st3[:, 0, i], in_=h3f[:, i * TCH:(i + 1) * TCH])
        nc.vector.bn_aggr(out=mv3[:, 0], in_=st3[:, 0])
        stats3 = small.tile([128, 1, 2], F32, tag="bnpk3")
        m3b = small.tile([128, 1], F32, tag="bnm3")
        nc.vector.tensor_scalar(out=stats3[:, :, 0:1], in0=mv3[:, :, 0:1],
                                scalar1=float(T), scalar2=None, op0=ALU.mult)
        nc.vector.tensor_mul(out=m3b, in0=mv3[:, :, 0], in1=mv3[:, :, 1 - 1])
        nc.vector.tensor_add(out=m3b, in0=m3b, in1=mv3[:, :, 1])
        nc.vector.tensor_scalar(out=stats3[:, :, 1:2], in0=m3b.rearrange("p (e o) -> p e o", o=1),
                                scalar1=float(T), scalar2=None, op0=ALU.mult)
        nc.gpsimd.dma_start(out=ar_in[3][:], in_=stats3.rearrange("p e two -> p (e two)"))
        nc.gpsimd.collective_compute("AllReduce", ALU.add, RG,
                                     ins=[ar_in[3][:]], outs=[ar_out[3][:]])
        g3 = small.tile([128, 1, 2], F32, tag="bngl3")
        nc.gpsimd.dma_start(out=g3.rearrange("p e two -> p (e two)"), in_=ar_out[3][:])
        a3, c3 = bn_affine(g3, 1, bn2_g, bn2_b)

        nc.vector.tensor_scalar(out=h3f, in0=h3f, scalar1=a3, scalar2=c3,
                                op0=ALU.mult, op1=ALU.add)
        nc.vector.tensor_add(out=x_mhsa, in0=x_mhsa, in1=h3s)
        nc.sync.dma_start(out=out_t[:].rearrange("b c h w -> c b (h w)"), in_=x_mhsa)


_cached = None


def kernel(**inputs):
    global last_result, _cached
    hp = _host_prep(inputs)
    ln_triv = hp.pop("_ln_triv")
    if _cached is None or _cached[1] != ln_triv:
        _cached = (_build(ln_triv), ln_triv)
    nc = _cached[0]
    x = np.ascontiguousarray(np.asarray(inputs["x"], dtype=np.float32))
    in_maps = []
    for c in range(NC):
        m = dict(hp)
        m["xs"] = np.ascontiguousarray(x[c * BL:(c + 1) * BL])
        in_maps.append(m)
    trace = os.environ.get("KERNEL_TRACE", "0") == "1"
    res = run_bass_kernel_spmd(nc, in_maps, core_ids=list(range(NC)), trace=trace)
    last_result = res
    return np.concatenate([r["out"] for r in res.results], axis=0)



# revision 42
# speedup vs baseline: 1.1335x; 1.0808x over previous
import os, sys
import numpy as np

sys.path.insert(0, "/opt/trn_rl_repo")

import concourse.bass as bass
import concourse.bacc as bacc
import concourse.tile as tile
import concourse.mybir as mybir
from concourse.bass_utils import run_bass_kernel_spmd

F32 = mybir.dt.float32
F32R = mybir.dt.float32r
BF16 = mybir.dt.bfloat16
F8 = mybir.dt.float8e4
AF = mybir.ActivationFunctionType
ALU = mybir.AluOpType

NC = 8
B, C, H, W = 64, 128, 28, 28
BL = B // NC
HW = H * W
T = BL * HW                  # 6272
HEADS, D = 4, 32
E = 512
KV, L = 15, 225
EPS = 1e-5
NG = float(B * HW)
SCALE = D ** -0.5
NT, TCH = 14, 448
KC0, KC1 = 128, L - 128

last_result = None


def _f32r(ap):
    return ap.bitcast(F32R)


def _class_ranges(k):
    if k == 0:
        return (1, 2)
    if k == 1:
        return (0, 1, 2)
    return (0, 1)


def _host_prep(inputs):
    import ml_dtypes
    bf = ml_dtypes.bfloat16
    f = lambda a: np.ascontiguousarray(np.asarray(a), dtype=np.float32)
    inp = {k: np.asarray(v) for k, v in inputs.items()}
    h = {}

    def diag(wk, ntap, dt):
        ch = wk.shape[0]
        nch = ch // 128
        out = np.zeros((128, nch, ntap, 128), dtype=np.float32)
        for cc in range(nch):
            for t in range(ntap):
                out[np.arange(128), cc, t, np.arange(128)] = wk[cc * 128:(cc + 1) * 128, t]
        return np.ascontiguousarray(out.astype(dt))

    import ml_dtypes as mld0
    f8t = mld0.float8_e4m3

    def diag_pairs(wk, pairs, dt, scale=16.0):
        # wk [ch, ntap]; returns [128, nch, npair, 2, 128]
        ch = wk.shape[0]
        nch = ch // 128
        out = np.zeros((128, nch, len(pairs), 2, 128), dtype=np.float32)
        for cc in range(nch):
            for pi, (ta, tb) in enumerate(pairs):
                out[np.arange(128), cc, pi, 0, np.arange(128)] = wk[cc * 128:(cc + 1) * 128, ta] * scale
                out[np.arange(128), cc, pi, 1, np.arange(128)] = wk[cc * 128:(cc + 1) * 128, tb] * scale
        return np.ascontiguousarray(out.astype(dt))

    # 3x3 taps indexed kh*3+kw; pairs chosen with constant in-pad stride:
    # (0,0)+(0,2)->S2, (2,0)+(2,2)->S2, (0,1)+(2,1)->S60, (1,0)+(1,2)->S2
    P9 = [(0, 2), (6, 8), (1, 7), (3, 5)]
    lpu9 = f(inp["lpu_w"]).reshape(C, 9)
    h["lpur"] = diag_pairs(lpu9, P9, f8t).reshape(128, 4, 2, 128)
    h["lpuc"] = diag((lpu9[:, 4:5] * 16.0), 1, f8t).reshape(128, 128)
    # 2x2 stride-2 taps kh*2+kw; pairs (0,0)+(0,1)->S1, (1,0)+(1,1)->S1
    P4 = [(0, 1), (2, 3)]
    h["kdwr"] = diag_pairs(f(inp["kdw_w"]).reshape(C, 4), P4, f8t).reshape(128, 2, 2, 128)
    h["vdwr"] = diag_pairs(f(inp["vdw_w"]).reshape(C, 4), P4, f8t).reshape(128, 2, 2, 128)
    h["wqT"] = f(inp["wq"]).T.copy().astype(bf)
    h["wkT"] = f(inp["wk"]).T.copy().astype(bf)
    h["wvT"] = f(inp["wv"]).T.copy().astype(bf)
    h["woT"] = f(inp["wo"]).T.copy().astype(bf)
    h["bq"] = f(inp["bq"]).reshape(C, 1)
    h["bkp"] = (f(inp["bk"]) + f(inp["wk"]) @ f(inp["kdw_b"])).reshape(C, 1)
    bvp = f(inp["bv"]) + f(inp["wv"]) @ f(inp["vdw_b"])
    h["bop"] = (f(inp["bo"]) + f(inp["wo"]) @ bvp + f(inp["lpu_b"])).reshape(C, 1)
    import ml_dtypes as mld
    f8 = mld.float8_e4m3
    # raw attention bias (divided by softmax scale), keys-major, for PSUM
    # preload ahead of the QK matmul: et = exp(SCALE*(qk + bias/SCALE))
    bq_ = f(inp["attn_bias"])[0].transpose(0, 2, 1) / (D ** -0.5)  # [4, 225, 784]
    bqp = np.zeros((128, 2, HEADS, HW), dtype=np.float32)
    bqp[:, 0] = bq_[:, 0:128, :].transpose(1, 0, 2)
    bqp[:KC1, 1] = bq_[:, 128:L, :].transpose(1, 0, 2)
    h["biasq"] = np.ascontiguousarray(bqp.astype(f8))
    eb = np.exp(f(inp["attn_bias"]))[0].transpose(0, 2, 1)  # [4, 225, 784]
    ebp = np.zeros((128, 2, HEADS, HW), dtype=np.float32)
    ebp[:, 0] = eb[:, 0:128, :].transpose(1, 0, 2)
    ebp[:KC1, 1] = eb[:, 128:L, :].transpose(1, 0, 2)
    h["expb"] = np.ascontiguousarray(ebp.astype(f8))
    eb = np.exp(f(inp["attn_bias"]))[0].transpose(0, 2, 1)  # [4, 225, 784]
    ebp = np.zeros((128, 2, HEADS, HW), dtype=np.float32)
    ebp[:, 0] = eb[:, 0:128, :].transpose(1, 0, 2)
    ebp[:KC1, 1] = eb[:, 128:L, :].transpose(1, 0, 2)
    h["expb"] = np.ascontiguousarray(ebp.astype(f8))
    idp = np.zeros((128, 128), dtype=np.float32)
    idp[np.arange(128), np.arange(128)] = 1.0
    h["identp"] = np.ascontiguousarray(idp.astype(f8))
    dww = f(inp["dw_w"]).reshape(E, 3, 3).copy()
    dww[:, 1, 1] += 1.0
    dwr9 = f(inp["dw_w"]).reshape(E, 9)
    h["fdr"] = diag_pairs(dwr9, P9, f8t)                 # [128, 4, 4, 2, 128]
    h["fctr"] = diag((dwr9[:, 4:5] + 1.0) * 16.0, 1, bf).reshape(128, 4, 128)
    h["dw_b"] = f(inp["dw_b"]).reshape(4, 128).T.copy()
    psum9 = np.zeros((9, 4, 128), dtype=np.float32)
    for k in range(9):
        hr, wr = _class_ranges(k // 3), _class_ranges(k % 3)
        s = dww[:, hr, :][:, :, wr].sum(axis=(1, 2))
        psum9[k] = s.reshape(4, 128)
    h["psum9"] = (psum9 * 16.0).astype(bf)
    ind9 = np.zeros((9, H, W), dtype=np.float32)
    hc = np.full(H, 1); hc[0] = 0; hc[-1] = 2
    wc = np.full(W, 1); wc[0] = 0; wc[-1] = 2
    for i in range(H):
        for j in range(W):
            ind9[hc[i] * 3 + wc[j], i, j] = 1.0
    h["ind9"] = ind9.reshape(9, HW).astype(bf)
    h["c1wT"] = f(inp["c1_w"]).T.copy().astype(bf)
    h["c1_b"] = f(inp["c1_b"]).reshape(4, 128).T.copy()
    h["w2T"] = f(inp["c2_w"]).T.reshape(4, 128, 128).transpose(1, 0, 2).copy().astype(bf)
    h["bn1_g"] = f(inp["bn1_g"]).reshape(4, 128).T.copy()
    h["bn1_b"] = f(inp["bn1_b"]).reshape(4, 128).T.copy()
    h["bnr_g"] = f(inp["bnr_g"]).reshape(4, 128).T.copy()
    h["bnr_b"] = f(inp["bnr_b"]).reshape(4, 128).T.copy()
    h["bn2_g"] = f(inp["bn2_g"]).reshape(C, 1)
    h["bn2_b"] = f(inp["bn2_b"]).reshape(C, 1)
    ln_triv = (np.allclose(inp["ln1_g"], 1) and np.allclose(inp["ln1_b"], 0)
               and np.allclose(inp["ln2_g"], 1) and np.allclose(inp["ln2_b"], 0))
    h["_ln_triv"] = ln_triv
    if not ln_triv:
        h["ln1_g"] = f(inp["ln1_g"]).reshape(1, HW)
        h["ln1_b"] = f(inp["ln1_b"]).reshape(1, HW)
        h["ln2_g"] = f(inp["ln2_g"]).reshape(1, HW)
        h["ln2_b"] = f(inp["ln2_b"]).reshape(1, HW)
    return h


def _build(ln_triv):
    nc = bacc.Bacc(None, target_bir_lowering=False, num_devices=NC)
    dt = nc.dram_tensor
    xs = dt("xs", [BL, C, H, W], F32, kind="ExternalInput")
    out_t = dt("out", [BL, C, H, W], F32, kind="ExternalOutput")
    hin = {}
    specs = [
        ("lpur", [128, 4, 2, 128], F8), ("lpuc", [128, 128], F8),
        ("kdwr", [128, 2, 2, 128], F8), ("vdwr", [128, 2, 2, 128], F8),
        ("fdr", [128, 4, 4, 2, 128], F8), ("fctr", [128, 4, 128], BF16),
        ("wqT", [C, C], BF16), ("wkT", [C, C], BF16), ("wvT", [C, C], BF16),
        ("woT", [C, C], BF16), ("bq", [C, 1], F32), ("bkp", [C, 1], F32),
        ("bop", [C, 1], F32), ("expb", [128, 2, HEADS, HW], F8),
        ("dw_b", [128, 4], F32),
        ("psum9", [9, 4, 128], BF16), ("ind9", [9, HW], BF16),
        ("c1wT", [C, E], BF16), ("c1_b", [128, 4], F32),
        ("w2T", [128, 4, 128], BF16),
        ("bn1_g", [128, 4], F32), ("bn1_b", [128, 4], F32),
        ("bnr_g", [128, 4], F32), ("bnr_b", [128, 4], F32),
        ("bn2_g", [C, 1], F32), ("bn2_b", [C, 1], F32),
    ]
    if not ln_triv:
        specs += [(n, [1, HW], F32) for n in ["ln1_g", "ln1_b", "ln2_g", "ln2_b"]]
    for name, shape, d in specs:
        hin[name] = dt(name, shape, d, kind="ExternalInput")
    ar_in = {0: dt("ar0i", [128, 1], F32, kind="Internal"),
             1: dt("ar1i", [128, 8], F32, kind="Internal"),
             2: dt("ar2i", [128, 8], F32, kind="Internal"),
             3: dt("ar3i", [128, 2], F32, kind="Internal")}
    ar_out = {0: dt("ar0o", [128, 1], F32, kind="Internal", addr_space="Shared"),
              1: dt("ar1o", [128, 8], F32, kind="Internal", addr_space="Shared"),
              2: dt("ar2o", [128, 8], F32, kind="Internal", addr_space="Shared"),
              3: dt("ar3o", [128, 2], F32, kind="Internal", addr_space="Shared")}
    c1_dram = dt("c1d", [128, 4], F32, kind="Internal")
    RG = [list(range(NC))]
    with tile.TileContext(nc) as tc:
        _emit(nc, tc, xs, out_t, hin, ar_in, ar_out, c1_dram, RG, ln_triv)
    if not nc.is_finalized():
        nc.finalize()
    return nc


def _emit(nc, tc, xs, out_t, hin, ar_in, ar_out, c1_dram, RG, ln_triv):
    from contextlib import ExitStack
    ctx = ExitStack()
    with ctx:
        big = ctx.enter_context(tc.tile_pool(name="big", bufs=2))
        bfp = ctx.enter_context(tc.tile_pool(name="bfp", bufs=1))
        cons = ctx.enter_context(tc.tile_pool(name="cons", bufs=1))
        small = ctx.enter_context(tc.tile_pool(name="small", bufs=1))
        etp = ctx.enter_context(tc.tile_pool(name="etp", bufs=8))
        psA = ctx.enter_context(tc.tile_pool(name="psA", bufs=4, space="PSUM"))
        psB = ctx.enter_context(tc.tile_pool(name="psB", bufs=2, space="PSUM"))
        psC = ctx.enter_context(tc.tile_pool(name="psC", bufs=2, space="PSUM"))

        def loadc(name):
            hh = hin[name]
            t = cons.tile(list(hh.shape), hh.dtype, tag=name)
            nc.gpsimd.dma_start(out=t, in_=hh[:])
            return t

        nc.gpsimd.collective_compute("AllReduce", ALU.add, RG,
                                     ins=[ar_in[0][:]], outs=[ar_out[0][:]])
        lpur = loadc("lpur"); lpuc = loadc("lpuc")
        kdwr = loadc("kdwr"); vdwr = loadc("vdwr")
        fdr = loadc("fdr"); fctr = loadc("fctr")
        wqT = loadc("wqT"); wkT = loadc("wkT"); wvT = loadc("wvT"); woT = loadc("woT")
        bq = loadc("bq"); bkp = loadc("bkp"); bop = loadc("bop")
        expbt = loadc("expb")
        dw_b = loadc("dw_b")
        psum9 = loadc("psum9"); ind9t = loadc("ind9")
        c1wT = loadc("c1wT"); c1_b = loadc("c1_b"); w2T = loadc("w2T")
        bn1_g = loadc("bn1_g"); bn1_b = loadc("bn1_b")
        bnr_g = loadc("bnr_g"); bnr_b = loadc("bnr_b")
        bn2_g = loadc("bn2_g"); bn2_b = loadc("bn2_b")
        ind9 = ind9t.rearrange("k (h w) -> k h w", h=H)
        lns = {}
        if not ln_triv:
            for nm in ["ln1_g", "ln1_b", "ln2_g", "ln2_b"]:
                t = cons.tile([128, HW], F32, tag=nm)
                nc.gpsimd.dma_start(out=t, in_=bass.AP(tensor=hin[nm], offset=0, ap=[[0, 128], [1, HW]]))
                lns[nm] = t
        epsT = small.tile([128, 1], F32, tag="epsT")
        nc.vector.memset(epsT, EPS)
        # pre-touch DMA-loaded consts on the engines that read them, so heavy
        # ops don't accumulate multiple DMA-queue sem waits (codegen limit)
        scrD = small.tile([128, 1], F32, tag="scrD")
        scrA = small.tile([128, 1], F32, tag="scrA")
        for t2 in (bq, bkp, bop, bn2_g, bn2_b):
            nc.vector.tensor_copy(out=scrD, in_=t2[:, 0:1])
        nc.vector.tensor_copy(out=scrD, in_=lpur[:, 0, 0, 0:1])
        nc.vector.tensor_copy(out=scrD, in_=kdwr[:, 0, 0, 0:1])
        nc.vector.tensor_copy(out=scrD, in_=vdwr[:, 0, 0, 0:1])
        nc.vector.tensor_copy(out=scrD, in_=w2T[:, 0, 0:1])
        nc.vector.tensor_copy(out=scrD, in_=lpuc[:, 0:1])
        nc.vector.tensor_copy(out=scrD, in_=fdr[:, 0, 0, 0, 0:1])
        nc.vector.tensor_copy(out=scrD, in_=fctr[:, 0, 0:1])
        for t4 in (wqT, wkT, wvT, woT, c1wT):
            nc.vector.tensor_copy(out=scrD, in_=t4[:, 0:1])
        nc.vector.tensor_copy(out=scrD, in_=expbt[:, 0, 0, 0:1])
        for t5 in (dw_b, c1_b, bn1_g, bn1_b, bnr_g, bnr_b):
            nc.vector.tensor_copy(out=scrD, in_=t5[:, 0:1])
        nc.vector.tensor_copy(out=scrD[0:9], in_=psum9[:, 0, 0:1])
        nc.vector.tensor_copy(out=scrD[0:9], in_=ind9t[:, 0:1])
        for t6 in lns.values():
            nc.vector.tensor_copy(out=scrD, in_=t6[:, 0:1])
        nc.scalar.mul(out=scrA, in_=c1_b[:, 0:1], mul=1.0)
        nc.scalar.mul(out=scrA, in_=dw_b[:, 0:1], mul=1.0)

        xsb = big.tile([128, BL, HW], F32, tag="big")
        nc.gpsimd.dma_start(out=xsb, in_=xs[:].rearrange("b c h w -> c b (h w)"))
        nc.vector.tensor_copy(out=scrD, in_=xsb[:, 0, 0:1])
        # zero-padded fp8 copy of x: [128, BL, 30, 30], image at [1:29, 1:29]
        xpad = bfp.tile([128, BL, 30, 30], F8, tag="pad8", bufs=2)
        nc.vector.memset(xpad[:, :, 0, :], 0.0)
        nc.vector.memset(xpad[:, :, 29, :], 0.0)
        nc.vector.memset(xpad[:, :, 1:29, 0], 0.0)
        nc.vector.memset(xpad[:, :, 1:29, 29], 0.0)
        with nc.allow_low_precision("conv input in fp8"):
            xsb4v = xsb.rearrange("p b (h w) -> p b h w", h=H)
            for b in range(BL):
                nc.vector.tensor_copy(out=xpad[:, b, 1:29, 1:29], in_=xsb4v[:, b])

        # x16-scaled fp8 DoubleRow taps; psum holds 16*dw(x); the +x residual
        # and /16 happen at evac.  Pair t reads (khA,kwA)/(khB,kwB) windows of
        # the padded image via an overlapping stride-S access pattern.
        P9T = [((0, 0), (0, 2)), ((2, 0), (2, 2)), ((0, 1), (2, 1)), ((1, 0), (1, 2))]

        def pad_pair_rhs(padt, pre, b, base, pa, pb_):
            (ka, wa), (kb, wb) = pa, pb_
            S = (kb - ka) * 30 + (wb - wa)
            off = padt.offset + (pre + b) * 900 + (base + ka) * 30 + wa
            return bass.AP(tensor=padt.tensor, offset=off,
                           ap=[list(padt.ap[0]), [S, 2], [30, 14], [1, 28]])

        def pad_tap_rhs(padt, pre, b, base, kh, kw):
            off = padt.offset + (pre + b) * 900 + (base + kh) * 30 + kw
            return bass.AP(tensor=padt.tensor, offset=off,
                           ap=[list(padt.ap[0]), [30, 14], [1, 28]])

        x_lpu = big.tile([128, BL, HW], F32, tag="big")
        xlp4 = x_lpu.rearrange("p b (h w) -> p b h w", h=H)
        xsb4 = xsb.rearrange("p b (h w) -> p b h w", h=H)

        for half in range(2):
            base = 14 * half
            pts = []
            for b in range(BL):
                pool = (psA, psA, psA, psA, psB, psB, psC, psC)[b]
                tg = ("mm", "mm", "mm", "mm", "dwp", "dwp", "av", "av")[b]
                pts.append(pool.tile([128, 14, W], F32, tag=tg, name=f"lvp{b}"))
            for pi in range(4):
                pa, pb_ = P9T[pi]
                for b in range(BL):
                    nc.tensor.matmul(
                        pts[b], lpur[:, pi], pad_pair_rhs(xpad, 0, b, base, pa, pb_),
                        start=(pi == 0), stop=False,
                        perf_mode=mybir.MatmulPerfMode.DoubleRow,
                        skip_group_check=True)
            for b in range(BL):
                nc.tensor.matmul(
                    pts[b], lpuc, pad_tap_rhs(xpad, 0, b, base, 1, 1),
                    start=False, stop=True, skip_group_check=True)
            for b in range(BL):
                nc.vector.scalar_tensor_tensor(
                    out=xlp4[:, b, base:base + 14, :], in0=pts[b], scalar=1.0 / 16.0,
                    in1=xsb4[:, b, base:base + 14, :], op0=ALU.mult, op1=ALU.add)

        # LN over HW
        def layer_norm(src, gname, dst):
            sv = src.rearrange("p b (two q) -> p b two q", two=2)
            st = small.tile([128, BL, 2, 6], F32, tag="lnst")
            mv = small.tile([128, BL, 2], F32, tag="lnmv")
            sd = small.tile([128, BL, 1], F32, tag="lnsd")
            for b in range(BL):
                for g2 in range(2):
                    nc.vector.bn_stats(out=st[:, b, g2], in_=sv[:, b, g2])
                nc.vector.bn_aggr(out=mv[:, b], in_=st[:, b])
            nc.scalar.activation(out=sd, in_=mv[:, :, 1:2], func=AF.Sqrt, bias=epsT, scale=1.0)
            nc.vector.reciprocal(out=sd, in_=sd)
            for b in range(BL):
                nc.vector.tensor_scalar(
                    out=dst[:, b], in0=src[:, b], scalar1=mv[:, b, 0:1], scalar2=sd[:, b],
                    op0=ALU.subtract, op1=ALU.mult)
            if not ln_triv:
                g = lns[gname + "_g"]; bb = lns[gname + "_b"]
                for b in range(BL):
                    nc.vector.tensor_mul(out=dst[:, b], in0=dst[:, b], in1=g)
                    nc.vector.tensor_add(out=dst[:, b], in0=dst[:, b], in1=bb)

        xnbf = bfp.tile([128, BL, HW], BF16, tag="t12b")
        layer_norm(x_lpu, "ln1", xnbf)
        xnpad = bfp.tile([128, BL, 30, 30], F8, tag="pad8", bufs=2)
        nc.vector.memset(xnpad[:, :, 0, :], 0.0)
        nc.vector.memset(xnpad[:, :, 29, :], 0.0)
        nc.vector.memset(xnpad[:, :, 1:29, 0], 0.0)
        nc.vector.memset(xnpad[:, :, 1:29, 29], 0.0)
        with nc.allow_low_precision("conv input in fp8"):
            xnbf4v = xnbf.rearrange("p b (h w) -> p b h w", h=H)
            for b in range(BL):
                nc.vector.tensor_copy(out=xnpad[:, b, 1:29, 1:29], in_=xnbf4v[:, b])

        # Q projection (f32r) -> bf16
        qbf = bfp.tile([128, BL, HW], BF16, tag="qbf")
        xnbff = xnbf.rearrange("p b q -> p (b q)")
        qbff = qbf.rearrange("p b q -> p (b q)")
        for i in range(NT):
            pt = psA.tile([128, TCH], F32, tag="mm")
            nc.tensor.matmul(pt, wqT, xnbff[:, i * TCH:(i + 1) * TCH], start=True, stop=True)
            nc.vector.tensor_scalar(out=qbff[:, i * TCH:(i + 1) * TCH], in0=pt, scalar1=bq,
                                    scalar2=None, op0=ALU.add)
        # K/V strided 2x2 dw conv
        kxbf = bfp.tile([128, BL, L], BF16, tag="kxbf")
        vxbf = bfp.tile([128, BL, L], BF16, tag="vxbf")
        kx4 = kxbf.rearrange("p b (i j) -> p b i j", i=KV)
        vx4 = vxbf.rearrange("p b (i j) -> p b i j", i=KV)
        # kv conv: out(i,j) = sum w[kh,kw]*xn[2i+kh-1, 2j+kw-1]
        #        = sum w[kh,kw]*xnpad[2i+kh, 2j+kw], i,j in [0,15)
        def kv_pair_rhs(b, kh):
            off = xnpad.offset + b * 900 + kh * 30
            return bass.AP(tensor=xnpad.tensor, offset=off,
                           ap=[list(xnpad.ap[0]), [1, 2], [60, KV], [2, KV]])

        for b in range(BL):
            for dst4, dg in ((kx4, kdwr), (vx4, vdwr)):
                pt = psA.tile([128, KV, KV], F32, tag="mm")
                for kh in range(2):
                    nc.tensor.matmul(
                        pt, dg[:, kh], kv_pair_rhs(b, kh),
                        start=(kh == 0), stop=(kh == 1),
                        perf_mode=mybir.MatmulPerfMode.DoubleRow,
                        skip_group_check=True)
                nc.scalar.activation(out=dst4[:, b], in_=pt, func=AF.Copy,
                                     scale=1.0 / 16.0)
        kbf = bfp.tile([128, BL, L], BF16, tag="kbf")
        kxf = kxbf.rearrange("p b l -> p (b l)")
        kbff = kbf.rearrange("p b l -> p (b l)")
        for i in range(4):
            pt = psA.tile([128, 450], F32, tag="mm")
            nc.tensor.matmul(pt, wkT, kxf[:, i * 450:(i + 1) * 450], start=True, stop=True)
            nc.vector.tensor_scalar(out=kbff[:, i * 450:(i + 1) * 450], in0=pt, scalar1=bkp,
                                    scalar2=None, op0=ALU.add)
        vaug = bfp.tile([128, BL, 2, HEADS, 64], F8, tag="vaug")
        nc.vector.memset(vaug, 0.0)
        nc.vector.memset(vaug[:, :, :, :, 32:64], 1.0)
        for b in range(BL):
            for kc in range(2):
                ktM = KC0 if kc == 0 else KC1
                pt = psA.tile([128, 128], F32, tag="mm")
                nc.tensor.matmul(pt[0:ktM], vxbf[:, b, kc * 128: kc * 128 + ktM], wvT,
                                 start=True, stop=True)
                with nc.allow_low_precision("attention V in fp8"):
                    nc.scalar.copy(out=vaug[0:ktM, b, kc, :, 0:32],
                                   in_=pt[0:ktM].rearrange("p (h d) -> p h d", h=HEADS))

        # attention: QK into psum (4 heads row-tiled), then the attention bias
        # preloaded on top via one full identity matmul; exp at evac includes
        # the bias; AV with a ones-block for denominators; fast-approx
        # reciprocal of the denominators.
        o_sb = bfp.tile([128, BL, HW], BF16, tag="t12a")
        rbc = bfp.tile([128, BL, HW], F32, tag="h1h2")
        for b in range(BL):
            dscr = bfp.tile([128, HW], F32, tag="kxbf", name=f"dscr{b}")
            for qc in range(2):
                qs = slice(qc * 392, (qc + 1) * 392)
                ets = {}
                for kc in range(2):
                    ktM = KC0 if kc == 0 else KC1
                    pts = []
                    for hd in range(HEADS):
                        pt = psA.tile([128, 392], F32, tag="mm")
                        nc.tensor.matmul(
                            pt[0:ktM],
                            kbf[hd * 32:(hd + 1) * 32, b, kc * 128: kc * 128 + ktM],
                            qbf[hd * 32:(hd + 1) * 32, b, qs],
                            start=True, stop=True, tile_position=(hd * 32, 0))
                        pts.append(pt)
                    for hd in range(HEADS):
                        pt = pts[hd]
                        et = etp.tile([128, 392], F8, tag="et")
                        with nc.allow_low_precision("attention scores fp8"):
                            nc.scalar.activation(out=et[0:ktM], in_=pt[0:ktM],
                                                 func=AF.Exp, scale=SCALE)
                            nc.vector.tensor_mul(out=et[0:ktM], in0=et[0:ktM],
                                                 in1=expbt[0:ktM, kc, hd, qs])
                        ets[(kc, hd)] = et
                for hp in range(2):
                    pv = psC.tile([128, 392], F32, tag="av")
                    for kc in range(2):
                        ktM = KC0 if kc == 0 else KC1
                        for hh in range(2):
                            hd = hp * 2 + hh
                            nc.tensor.matmul(
                                pv[64 * hh:64 * hh + 64],
                                vaug[0:ktM, b, kc, hd, :], ets[(kc, hd)][0:ktM],
                                start=(kc == 0), stop=(kc == 1),
                                tile_position=(0, 64 * hh), skip_group_check=True)
                    for hh in range(2):
                        hd = hp * 2 + hh
                        with nc.allow_low_precision("attention numerators bf16"):
                            nc.vector.tensor_copy(
                                out=o_sb[hd * 32:(hd + 1) * 32, b, qs],
                                in_=pv[64 * hh:64 * hh + 32])
                        nc.scalar.copy(
                            out=dscr[hd * 32:(hd + 1) * 32, qs],
                            in_=pv[64 * hh + 32:64 * hh + 64])
                # fast reciprocal of this (b, qc)'s denominators (packed dup
                # layout): magic seed + one Newton, standard ops.  rbc ends
                # NEGATED ((x*y0-2)*y0 = -1/x); the o-mul flips the sign.
                nc.vector.tensor_scalar(
                    out=rbc[:, b, qs].bitcast(mybir.dt.int32),
                    in0=dscr[:, qs].bitcast(mybir.dt.int32),
                    scalar1=-1, scalar2=0x7EF127EA, op0=ALU.mult, op1=ALU.add)
                nc.vector.tensor_mul(out=dscr[:, qs], in0=dscr[:, qs], in1=rbc[:, b, qs])
                nc.vector.scalar_tensor_tensor(
                    out=rbc[:, b, qs], in0=dscr[:, qs], scalar=2.0, in1=rbc[:, b, qs],
                    op0=ALU.subtract, op1=ALU.mult)
        nc.vector.scalar_tensor_tensor(out=o_sb, in0=o_sb, scalar=-1.0, in1=rbc,
                                       op0=ALU.mult, op1=ALU.mult)

        x_mhsa = big.tile([128, BL, HW], F32, tag="big")
        of = o_sb.rearrange("p b q -> p (b q)")
        xmf = x_mhsa.rearrange("p b q -> p (b q)")
        xlf = x_lpu.rearrange("p b q -> p (b q)")
        for i in range(NT):
            pt = psA.tile([128, TCH], F32, tag="mm")
            nc.tensor.matmul(pt, woT, of[:, i * TCH:(i + 1) * TCH], start=True, stop=True)
            nc.vector.scalar_tensor_tensor(out=xmf[:, i * TCH:(i + 1) * TCH], in0=pt, scalar=bop,
                                           in1=xlf[:, i * TCH:(i + 1) * TCH], op0=ALU.add, op1=ALU.add)

        ybf = bfp.tile([128, BL, HW], BF16, tag="t12a")
        layer_norm(x_mhsa, "ln2", ybf)

        def bn_reduce(src_r, nchunk, ar_i, ar_o):
            # global batch statistics: local bn_stats/aggr, then a cross-core
            # AllReduce of (mean*T, (mean^2+var)*T)
            st = small.tile([128, nchunk, NT, 6], F32, tag="bnst")
            mv = small.tile([128, nchunk, 2], F32, tag="bnmv")
            for ecx in range(nchunk):
                for i in range(NT):
                    nc.vector.bn_stats(out=st[:, ecx, i], in_=src_r[:, ecx, i])
                nc.vector.bn_aggr(out=mv[:, ecx], in_=st[:, ecx])
            stats = small.tile([128, nchunk, 2], F32, tag="bnpack")
            m2 = small.tile([128, nchunk], F32, tag="bnm2")
            nc.vector.tensor_scalar(out=stats[:, :, 0:1], in0=mv[:, :, 0:1], scalar1=float(T),
                                    scalar2=None, op0=ALU.mult)
            nc.vector.tensor_mul(out=m2, in0=mv[:, :, 0], in1=mv[:, :, 0])
            nc.vector.tensor_add(out=m2, in0=m2, in1=mv[:, :, 1])
            nc.vector.tensor_scalar(out=stats[:, :, 1:2], in0=m2.rearrange("p (e o) -> p e o", o=1),
                                    scalar1=float(T), scalar2=None, op0=ALU.mult)
            nc.gpsimd.dma_start(out=ar_i[:], in_=stats.rearrange("p e two -> p (e two)"))
            nc.gpsimd.collective_compute("AllReduce", ALU.add, RG, ins=[ar_i[:]], outs=[ar_o[:]])
            g = small.tile([128, nchunk, 2], F32, tag="bngl")
            nc.gpsimd.dma_start(out=g.rearrange("p e two -> p (e two)"), in_=ar_o[:])
            return g

        def bn_affine(gs, nchunk, gt, bt, want_coa=False):
            a = small.tile([128, nchunk], F32, tag="bna")
            cc = small.tile([128, nchunk], F32, tag="bnc")
            mean = small.tile([128, nchunk], F32, tag="bnmean")
            m2 = small.tile([128, nchunk], F32, tag="bnm2b")
            nc.vector.tensor_scalar(out=mean, in0=gs[:, :, 0], scalar1=1.0 / NG, scalar2=None, op0=ALU.mult)
            nc.vector.tensor_scalar(out=a, in0=gs[:, :, 1], scalar1=1.0 / NG, scalar2=None, op0=ALU.mult)
            nc.vector.tensor_mul(out=m2, in0=mean, in1=mean)
            nc.vector.tensor_sub(out=a, in0=a, in1=m2)
            nc.scalar.activation(out=a, in_=a, func=AF.Sqrt, bias=epsT, scale=1.0)
            nc.vector.reciprocal(out=a, in_=a)
            nc.vector.tensor_mul(out=a, in0=a, in1=gt)
            nc.vector.tensor_mul(out=cc, in0=mean, in1=a)
            nc.vector.scalar_tensor_tensor(out=cc, in0=cc, scalar=-1.0, in1=bt,
                                           op0=ALU.mult, op1=ALU.add)
            if not want_coa:
                return a, cc
            ra = small.tile([128, nchunk], F32, tag="bnra")
            coa = small.tile([128, nchunk], F32, tag="bncoa")
            nc.vector.reciprocal(out=ra, in_=a)
            nc.vector.tensor_mul(out=coa, in0=cc, in1=ra)
            return a, cc, coa

        # pw1 + gelu -> h1bf
        h1bf = bfp.tile([128, 4, BL, HW], BF16, tag="h1h2")
        h1f = h1bf.rearrange("p e b q -> p e (b q)")
        ybff = ybf.rearrange("p b q -> p (b q)")
        for ec in range(4):
            for i in range(NT):
                pt = psA.tile([128, TCH], F32, tag="mm")
                nc.tensor.matmul(pt, c1wT[:, ec * 128:(ec + 1) * 128],
                                 ybff[:, i * TCH:(i + 1) * TCH], start=True, stop=True)
                nc.scalar.activation(out=h1f[:, ec, i * TCH:(i + 1) * TCH], in_=pt, func=AF.Gelu,
                                     bias=c1_b[:, ec:ec + 1], scale=1.0)
        # BN1: local stats feed the border-correction weights (lh9) so the
        # conv pipeline never waits on a collective; the gelu-evac scale/bias
        # (a1, via a1o16) uses the exact global stats from the AllReduce.
        st1l = small.tile([128, 4, NT, 6], F32, tag="bnst")
        mv1l = small.tile([128, 4, 2], F32, tag="bnmv")
        h1r = h1f.rearrange("p e (n q) -> p e n q", q=TCH)
        for ecx in range(4):
            for i in range(NT):
                nc.vector.bn_stats(out=st1l[:, ecx, i], in_=h1r[:, ecx, i])
            nc.vector.bn_aggr(out=mv1l[:, ecx], in_=st1l[:, ecx])
        stats1 = small.tile([128, 4, 2], F32, tag="bnpack")
        m2l = small.tile([128, 4], F32, tag="bnm2l")
        nc.vector.tensor_scalar(out=stats1[:, :, 0:1], in0=mv1l[:, :, 0:1],
                                scalar1=float(T), scalar2=None, op0=ALU.mult)
        nc.vector.tensor_mul(out=m2l, in0=mv1l[:, :, 0], in1=mv1l[:, :, 0])
        nc.vector.tensor_add(out=m2l, in0=m2l, in1=mv1l[:, :, 1])
        nc.vector.tensor_scalar(out=stats1[:, :, 1:2], in0=m2l.rearrange("p (e o) -> p e o", o=1),
                                scalar1=float(T), scalar2=None, op0=ALU.mult)
        nc.gpsimd.dma_start(out=ar_in[1][:], in_=stats1.rearrange("p e two -> p (e two)"))
        nc.gpsimd.collective_compute("AllReduce", ALU.add, RG,
                                     ins=[ar_in[1][:]], outs=[ar_out[1][:]])
        gs1l = small.tile([128, 4, 2], F32, tag="bngl1")
        nc.vector.tensor_scalar(out=gs1l, in0=stats1, scalar1=float(NG / T),
                                scalar2=None, op0=ALU.mult)
        _a1l, _c1l, coa1 = bn_affine(gs1l, 4, bn1_g, bn1_b, want_coa=True)
        nc.gpsimd.dma_start(out=c1_dram[:], in_=coa1)
        c1row = small.tile([1, 4, 128], F32, tag="c1row")
        nc.gpsimd.dma_start(out=c1row, in_=bass.AP(tensor=c1_dram, offset=0, ap=[[0, 1], [1, 4], [4, 128]]))
        c1f = small.tile([9, 4, 128], F32, tag="c1f")
        nc.gpsimd.partition_broadcast(c1f, c1row)
        lh9 = small.tile([9, 4, 128], BF16, tag="lh9")
        nc.vector.tensor_mul(out=lh9, in0=psum9, in1=c1f)
        # padded fp8 copies of h1, issued BEFORE anything that waits on the
        # BN1 AllReduce so the conv can start immediately
        h1b4 = h1bf.rearrange("p e b (h w) -> p e b h w", h=H)
        gf8s = []
        for ec in range(4):
            tg = "gf8" if ec < 2 else "pad8"
            gfp8 = bfp.tile([128, BL, 30, 30], F8, tag=tg, bufs=2, name=f"gch{ec}")
            nc.vector.memset(gfp8[:, :, 0, :], 0.0)
            nc.vector.memset(gfp8[:, :, 29, :], 0.0)
            nc.vector.memset(gfp8[:, :, 1:29, 0], 0.0)
            nc.vector.memset(gfp8[:, :, 1:29, 29], 0.0)
            with nc.allow_low_precision("conv input in fp8"):
                nc.vector.tensor_copy(
                    out=gfp8[:, :, 1:29, 1:29],
                    in_=h1b4[:, ec])
            gf8s.append(gfp8)
        g1 = small.tile([128, 4, 2], F32, tag="bngl")
        nc.gpsimd.dma_start(out=g1.rearrange("p e two -> p (e two)"), in_=ar_out[1][:])
        a1, c1 = bn_affine(g1, 4, bn1_g, bn1_b)
        a1o16 = small.tile([128, 4], F32, tag="a1o16")
        nc.vector.tensor_scalar(out=a1o16, in0=a1, scalar1=1.0 / 16.0, scalar2=None,
                                op0=ALU.mult)

        # FFN dw via x16 fp8 pairs + fp8 center + bf16 identity (the +h1
        # residual) + border-count correction; BN1 scale a1 applied at evac.
        # The padded fp8 copy of h1 is built per-ec (ring of 2) to fit SBUF.
        h2g = h1bf
        h2g4 = h2g.rearrange("p e b (h w) -> p e b h w", h=H)
        h2f = h2g.rearrange("p e b q -> p e (b q)")
        h2r = h2f.rearrange("p e (n q) -> p e n q", q=TCH)
        st2 = small.tile([128, 4, NT, 6], F32, tag="bnst")
        mv2 = small.tile([128, 4, 2], F32, tag="bnmv")
        for ec in range(4):
            gfp8 = gf8s[ec]
            for half in range(2):
                base = 14 * half
                pts = []
                for b in range(BL):
                    pool = (psA, psA, psA, psA, psB, psB, psC, psC)[b]
                    tg = ("mm", "mm", "mm", "mm", "dwp", "dwp", "av", "av")[b]
                    pts.append(pool.tile([128, 14, W], F32, tag=tg, name=f"cvp{b}"))
                for pi in range(4):
                    pa, pb_ = P9T[pi]
                    for b in range(BL):
                        nc.tensor.matmul(
                            pts[b], fdr[:, ec, pi],
                            pad_pair_rhs(gfp8, 0, b, base, pa, pb_),
                            start=(pi == 0), stop=False,
                            perf_mode=mybir.MatmulPerfMode.DoubleRow,
                            skip_group_check=True)
                for b in range(BL):
                    nc.tensor.matmul(
                        pts[b], fctr[:, ec], h1b4[:, ec, b, base:base + 14, :],
                        start=False, stop=False, skip_group_check=True)
                for b in range(BL):
                    nc.tensor.matmul(pts[b], lh9[:, ec], ind9[:, base:base + 14, :],
                                     start=False, stop=True, skip_group_check=True)
                for b in range(BL):
                    nc.scalar.activation(out=h2g4[:, ec, b, base:base + 14, :], in_=pts[b],
                                         func=AF.Gelu, bias=dw_b[:, ec:ec + 1],
                                         scale=a1o16[:, ec:ec + 1])
                if half == 1:
                    # per-ec stats as soon as this ec's evacs land, so only
                    # the AllReduce tail is exposed after the conv
                    for i in range(NT):
                        nc.vector.bn_stats(out=st2[:, ec, i], in_=h2r[:, ec, i])
                    nc.vector.bn_aggr(out=mv2[:, ec], in_=st2[:, ec])
        stats2 = small.tile([128, 4, 2], F32, tag="bnpack")
        m2b = small.tile([128, 4], F32, tag="bnm2")
        nc.vector.tensor_scalar(out=stats2[:, :, 0:1], in0=mv2[:, :, 0:1],
                                scalar1=float(T), scalar2=None, op0=ALU.mult)
        nc.vector.tensor_mul(out=m2b, in0=mv2[:, :, 0], in1=mv2[:, :, 0])
        nc.vector.tensor_add(out=m2b, in0=m2b, in1=mv2[:, :, 1])
        nc.vector.tensor_scalar(out=stats2[:, :, 1:2], in0=m2b.rearrange("p (e o) -> p e o", o=1),
                                scalar1=float(T), scalar2=None, op0=ALU.mult)
        nc.gpsimd.dma_start(out=ar_in[2][:], in_=stats2.rearrange("p e two -> p (e two)"))
        nc.gpsimd.collective_compute("AllReduce", ALU.add, RG,
                                     ins=[ar_in[2][:]], outs=[ar_out[2][:]])
        g2 = small.tile([128, 4, 2], F32, tag="bngl")
        nc.gpsimd.dma_start(out=g2.rearrange("p e two -> p (e two)"), in_=ar_out[2][:])
        a2, c2 = bn_affine(g2, 4, bnr_g, bnr_b)
        w2s = bfp.tile([128, 4, 128], BF16, tag="t12a")
        for kc in range(4):
            nc.vector.tensor_scalar(out=w2s[:, kc], in0=w2T[:, kc], scalar1=a2[:, kc:kc + 1],
                                    scalar2=None, op0=ALU.mult)
        c2bf = small.tile([128, 4], BF16, tag="c2bf")
        nc.vector.tensor_copy(out=c2bf, in_=c2)
        ptb = psC.tile([128, 1], F32, tag="av")
        for kc in range(4):
            nc.tensor.matmul(ptb, w2T[:, kc], c2bf[:, kc:kc + 1], start=(kc == 0), stop=(kc == 3))
        biasc = small.tile([128, 1], F32, tag="biascS")
        nc.vector.tensor_copy(out=biasc, in_=ptb)

        # pw2 -> h3s
        h3s = big.tile([128, BL, HW], F32, tag="big")
        h3f = h3s.rearrange("p b q -> p (b q)")
        st3 = small.tile([128, 1, NT, 6], F32, tag="bnst")
        mv3 = small.tile([128, 1, 2], F32, tag="bnmv")
        for i in range(NT):
            pt = psA.tile([128, TCH], F32, tag="mm")
            for kc in range(4):
                nc.tensor.matmul(pt, w2s[:, kc], h2f[:, kc, i * TCH:(i + 1) * TCH],
                                 start=(kc == 0), stop=(kc == 3))
            nc.vector.tensor_scalar(out=h3f[:, i * TCH:(i + 1) * TCH], in0=pt, scalar1=biasc,
                                    scalar2=None, op0=ALU.add)
            nc.vector.bn_stats(out=st3[:, 0, i], in_=h3f[:, i * TCH:(i + 1) * TCH])
        nc.vector.bn_aggr(out=mv3[:, 0], in_=st3[:, 0])
        stats3 = small.tile([128, 1, 2], F32, tag="bnpk3")
        m3b = small.tile([128, 1], F32, tag="bnm3")
        nc.vector.tensor_scalar(out=stats3[:, :, 0:1], in0=mv3[:, :, 0:1],
                                scalar1=float(T), scalar2=None, op0=ALU.mult)
        nc.vector.tensor_mul(out=m3b, in0=mv3[:, :, 0], in1=mv3[:, :, 1 - 1])
        nc.vector.tensor_add(out=m3b, in0=m3b, in1=mv3[:, :, 1])
        nc.vector.tensor_scalar(out=stats3[:, :, 1:2], in0=m3b.rearrange("p (e o) -> p e o", o=1),
                                scalar1=float(T), scalar2=None, op0=ALU.mult)
        nc.gpsimd.dma_start(out=ar_in[3][:], in_=stats3.rearrange("p e two -> p (e two)"))
        nc.gpsimd.collective_compute("AllReduce", ALU.add, RG,
                                     ins=[ar_in[3][:]], outs=[ar_out[3][:]])
        g3 = small.tile([128, 1, 2], F32, tag="bngl3")
        nc.gpsimd.dma_start(out=g3.rearrange("p e two -> p (e two)"), in_=ar_out[3][:])
        a3, c3 = bn_affine(g3, 1, bn2_g, bn2_b)

        nc.vector.tensor_scalar(out=h3f, in0=h3f, scalar1=a3, scalar2=c3,
                                op0=ALU.mult, op1=ALU.add)
        nc.vector.tensor_add(out=x_mhsa, in0=x_mhsa, in1=h3s)
        nc.sync.dma_start(out=out_t[:].rearrange("b c h w -> c b (h w)"), in_=x_mhsa)


_cached = None


def kernel(**inputs):
    global last_result, _cached
    hp = _host_prep(inputs)
    ln_triv = hp.pop("_ln_triv")
    if _cached is None or _cached[1] != ln_triv:
        _cached = (_build(ln_triv), ln_triv)
    nc = _cached[0]
    x = np.ascontiguousarray(np.asarray(inputs["x"], dtype=np.float32))
    in_maps = []
    for c in range(NC):
        m = dict(hp)
        m["xs"] = np.ascontiguousarray(x[c * BL:(c + 1) * BL])
        in_maps.append(m)
    trace = os.environ.get("KERNEL_TRACE", "0") == "1"
    res = run_bass_kernel_spmd(nc, in_maps, core_ids=list(range(NC)), trace=trace)
    last_result = res
    return np.concatenate([r["out"] for r in res.results], axis=0)

